# revision 1
# baseline (speedup 1.0000x reference)
"""Causal multi-head self-attention on 8 Trainium2 NeuronCores.

Problem: x[4,2048,1024], Wq/Wk/Wv/Wo[1024,1024], H=16 heads, dk=64.
  q = x@Wq.T, k = x@Wk.T, v = x@Wv.T  (per-head causal softmax(q k^T/8) v) @ Wo.T

Sharding: core c handles batch b=c//2 and head-half hh=c%2 (8 heads).
Each core returns a partial output (its 512 attn columns through the
matching 512 rows of Wo.T); the host sums core pairs.

Kernel layouts (all contractions on the partition axis):
  xT  [1024, 2048]  (d, s)      qT/kT [512, 2048] (head*64+dk, s)
  v   [s-block, head, 65]       (col 64 = ones -> softmax denominator)
  scoresT [k, q] blocks; exp on ACT (scale=1/8, no max-subtraction --
  scores are O(1) here); causal = block skip at 512-col granularity,
  column trim + affine_select zero-fill on diagonal blocks; attnT
  accumulated in PSUM with the ones column giving the denominator;
  normalization via DVE reciprocal + a rank-1 PE broadcast matmul;
  O-projection from attnT layout.

All matmul operands are float32r (full PE rate at N>=256, ~tf32
accuracy).  The per-chunk pipeline interleaves the next chunk's
projections and the previous chunk's O-projection into the ACT-bound
attention phase as PE "fillers"; exp covers two k-blocks per
instruction; PV trails two steps behind the score matmuls across head
boundaries.  Startup warms the ACT exp table and the PE clock gate
under the input DMAs.
"""

import numpy as np

import concourse.bass as bass
import concourse.mybir as mybir
import concourse.tile as tile
from concourse.bass_utils import run_bass_kernel_spmd
from concourse.vector_clock import ScopedClock, VectorClock

B, S, D, H, DK = 4, 2048, 1024, 16, 64
HPC = H // 2          # heads per core
HD = HPC * DK         # 512 head-dim columns per core
CH = 512              # q-chunk width
NCH = S // CH         # 4
NKB = S // 128        # 16 k-blocks
F32 = mybir.dt.float32
F32R = mybir.dt.float32r
EXP = mybir.ActivationFunctionType.Exp


def _drain_and_barrier_split(self, tick_clock, wait_clock):
    # The stock Tile tail drain attaches every outstanding sem wait to one
    # Drain instruction; this walrus build caps sync waits per instruction
    # and rejects it.  Put each wait on its own SP nop first, then drain
    # with no waits (SP has observed everything by then).
    gc = tick_clock.global_clock
    n = len(gc)
    for proc in range(n):
        t = gc[proc]
        if t == 0:
            continue
        vc = VectorClock([0] * n)
        vc.require_at_least(proc, t)
        nop = self.nc.sync.nop(nofuse=True)
        wait_clock.add_sem_waits(nop.ins, ScopedClock({None: vc}))
    self.nc.sync.drain()
    self.nc.all_engine_barrier()
    assert self.sems is not None
    popped = self.nc._tile_sem_poison_stack.pop()
    assert popped is self._sem_poison
    self.nc.clear_and_free_semaphores(list(self.sems.allocated().values()))
    self.nc.all_engine_barrier()


def _build_kernel(ctx, tc, xT, wqT, wkT, wvT, woT, out):
    nc = tc.nc
    KC = D // 128  # 8 contraction chunks for the projections

    wpool = ctx.enter_context(tc.tile_pool(name="weights", bufs=1))
    kvpool = ctx.enter_context(tc.tile_pool(name="kv", bufs=1))
    xpool = ctx.enter_context(tc.tile_pool(name="x", bufs=1))
    qpool = ctx.enter_context(tc.tile_pool(name="q", bufs=2))
    epool = ctx.enter_context(tc.tile_pool(name="exp", bufs=4))
    apool = ctx.enter_context(tc.tile_pool(name="attn", bufs=2))
    opool = ctx.enter_context(tc.tile_pool(name="osb", bufs=4))
    rpool = ctx.enter_context(tc.tile_pool(name="recip", bufs=2))
    # One PSUM pool, 8 banks: sc 2x[128,1024] (4) + at 3x[65,512] (3) +
    # bc 1x[64,512] (1).  Projection/O-proj groups share the "sc" slots.
    pp = ctx.enter_context(tc.tile_pool(name="pp", bufs=2, space="PSUM"))

    # --- whole-kernel-resident tiles ---
    wq = [wpool.tile([128, HD], F32R, tag=f"wq{kc}", name=f"wq{kc}")
          for kc in range(KC)]
    wk = [wpool.tile([128, HD], F32R, tag=f"wk{kc}", name=f"wk{kc}")
          for kc in range(KC)]
    wv = [wpool.tile([128, HD], F32R, tag=f"wv{kc}", name=f"wv{kc}")
          for kc in range(KC)]
    wo = wpool.tile([128, 4, D], F32R, tag="wo")
    ones = wpool.tile([1, DK], F32R, tag="ones")
    kT = kvpool.tile([128, 4, S], F32R, tag="kT")
    v = kvpool.tile([128, NKB, HPC, DK + 1], F32R, tag="v")

    def dma_x(j):
        cs = slice(j * CH, (j + 1) * CH)
        xch = [xpool.tile([128, CH], F32R, tag=f"x{kc}", name=f"x{kc}")
               for kc in range(KC)]
        for kc in range(KC):
            nc.sync.dma_start(out=xch[kc], in_=xT[kc * 128:(kc + 1) * 128, cs])
        return xch

    def dma_w(w, wT):
        for kc in range(KC):
            nc.sync.dma_start(out=w[kc], in_=wT[kc * 128:(kc + 1) * 128, :])

    # First matmul needs only xch0[0] + wq[0]: interleave those DMAs first.
    cs0 = slice(0, CH)
    xch0 = [xpool.tile([128, CH], F32R, tag=f"x{kc}", name=f"x{kc}")
            for kc in range(KC)]
    for kc in range(KC):
        xe = nc.sync if kc % 2 == 0 else nc.scalar
        xe.dma_start(out=xch0[kc], in_=xT[kc * 128:(kc + 1) * 128, cs0])
        nc.gpsimd.dma_start(out=wq[kc], in_=wqT[kc * 128:(kc + 1) * 128, :])
    dma_w(wk, wkT)
    dma_w(wv, wvT)
    nc.sync.dma_start(out=wo, in_=woT.rearrange("(c p) n -> p c n", p=128))
    ones_f32 = wpool.tile([1, DK], F32, tag="ones_f32")
    nc.vector.memset(ones_f32, 1.0)
    nc.vector.tensor_copy(ones, ones_f32)
    vcol_f32 = wpool.tile([128, NKB, HPC, 1], F32, tag="vcol_f32")
    nc.vector.memset(vcol_f32, 1.0)
    nc.vector.tensor_copy(v[:, :, :, DK:DK + 1], vcol_f32)
    warm = wpool.tile([128, 128], F32R, tag="warm")
    warm_f32 = wpool.tile([128, 128], F32, tag="warm_f32")
    nc.vector.memset(warm_f32, 0.0)
    nc.vector.tensor_copy(warm, warm_f32)
    # preload the ACT exp table set under the input DMAs (~2.7us on HW)
    rcw = rpool.tile([1, DK], F32, tag="rc", name="rcw", bufs=1)
    nc.scalar.activation(out=rcw, in_=ones_f32, func=EXP, scale=1.0)
    # hold the PE clock-gate open / absorb the cold ramp while DMAs land
    wps = pp.tile([128, 2 * CH], F32, tag="sc", name="wps")
    for r in range(12):
        nc.tensor.matmul(wps[:, (r % 2) * CH:(r % 2) * CH + 128],
                         lhsT=warm, rhs=warm, start=True, stop=True)

    def qkv_fillers(j, xch):
        cs = slice(j * CH, (j + 1) * CH)
        qch = qpool.tile([128, 4, CH], F32R, name=f"qch{j}", tag="qch")
        fillers = []
        dense = j == 0  # attention not running yet: borrow the sc slots

        def proj(w, dst, mb, nmb):
            def f():
                tag = "sc" if dense else "fill"
                ps = pp.tile([128, nmb * CH], F32, tag=tag, bufs=None if dense
                             else 1, name="psf")
                for t in range(nmb):
                    for kc in range(KC):
                        nc.tensor.matmul(
                            ps[:, t * CH:(t + 1) * CH],
                            lhsT=w[kc][:, (mb + t) * 128:(mb + t + 1) * 128],
                            rhs=xch[kc], start=(kc == 0), stop=(kc == KC - 1))
                nc.vector.tensor_copy(dst, ps)
            return f

        def vproj(sb, nsb):
            def f():
                tag = "sc" if dense else "fill"
                ps = pp.tile([128, nsb * CH], F32, tag=tag, bufs=None if dense
                             else 1, name="psf")
                for t in range(nsb):
                    for kc in range(KC):
                        nc.tensor.matmul(
                            ps[:, t * CH:(t + 1) * CH],
                            lhsT=xch[kc][:, (sb + t) * 128:(sb + t + 1) * 128],
                            rhs=wv[kc], start=(kc == 0), stop=(kc == KC - 1))
                sblk = j * 4 + sb
                nc.vector.tensor_copy(
                    v[:, sblk:sblk + nsb, :, 0:DK],
                    ps.rearrange("p (t h d) -> p t h d", t=nsb, h=HPC))
            return f

        nm = 2 if dense else 1
        for mb in range(0, 4, nm):
            fillers.append(proj(wq, qch[:, mb:mb + nm, :], mb, nm))
        kv = []  # (deadline (h, g) in chunk j's own attention loop, fn)
        for mb in range(0, 4, nm):
            # kT m-block mb is first read by head 2*mb at its step g=2j
            kv.append(((2 * mb, 2 * j - 1),
                       proj(wk, kT[:, mb:mb + nm, cs], mb, nm)))
        for sb in range(0, 4, nm):
            # v s-block 4j+sb is first read by the pv pair emitted at
            # step g = 2j + sb//2 + 2 of head 0
            kv.append(((0, 2 * j + sb // 2 + 1), vproj(sb, nm)))
        return qch, fillers, kv

    def o_fillers(j, ach):
        def oblk(sb, n):
            def f():
                sblk = j * 4 + sb
                osb = opool.tile([128, CH], F32, name="osb", tag="osb")
                ps = pp.tile([128, CH], F32, tag="fill", bufs=1, name="psf")
                for hp in range(4):
                    nc.tensor.matmul(
                        ps, lhsT=ach[:, hp, sb * 128:(sb + 1) * 128],
                        rhs=wo[:, hp, n * CH:(n + 1) * CH],
                        start=(hp == 0), stop=(hp == 3))
                nc.vector.tensor_copy(osb, ps)
                nc.sync.dma_start(
                    out=out[sblk * 128:(sblk + 1) * 128,
                            n * CH:(n + 1) * CH], in_=osb)
            return f
        return [oblk(sb, n) for sb in range(4) for n in range(2)]

    pending_norm = []

    def _norm_one(at_ps, dst):
        # Normalize a finished head: recip of the denominator row, rank-1
        # PE broadcast across the 64 dk partitions, multiply into attnT.
        rc = rpool.tile([1, CH], F32R, name="rc", tag="rc", bufs=1)
        with nc.allow_low_precision(reason="f32r feed for PE broadcast"):
            nc.vector.reciprocal(out=rc, in_=at_ps[DK:DK + 1, :])
        bc = pp.tile([DK, CH], F32, tag="at", bufs=3, name="bc")
        nc.tensor.matmul(bc, lhsT=ones, rhs=rc, start=True, stop=True)
        bcs = rpool.tile([DK, CH], F32, tag="bcs", bufs=1, name="bcs")
        nc.vector.tensor_copy(bcs, bc)
        nc.vector.tensor_mul(dst, at_ps[0:DK, :], bcs)

    from collections import deque
    fillers = deque()   # (None, fn) or ("next", (h, g), fn)
    carry_kv = deque()  # K/V fillers deferred into the current chunk
    carry_next = deque()
    qch, f0, kv0 = qkv_fillers(0, xch0)
    for f in f0:
        f()  # nothing to overlap with at the very start
    for _, f in kv0:
        f()

    prev = None  # (j, ach) of the chunk awaiting its O-projection
    for j in range(NCH):
        # stage next chunk's x DMAs + projection fillers, and the previous
        # chunk's O-projection, to fill PE gaps in this ACT-bound phase
        if prev is not None:
            fillers.extend((None, f) for f in o_fillers(*prev))
        if j + 1 < NCH:
            xch_n = dma_x(j + 1)
            qch_n, fs, kv_n = qkv_fillers(j + 1, xch_n)
            fillers.extend((None, f) for f in fs)
            fillers.extend((None, f) for _, f in kv_n)
        else:
            qch_n = None

        ach = apool.tile([128, 4, CH], F32R, name=f"ach{j}", tag="ach")
        nkb = 4 * (j + 1)
        steps = HPC * (nkb // 2)
        npop = 0
        nfill0 = len(fillers) + len(carry_kv)
        gstep = 0

        closed = set()

        def emit_pv(ent):
            at_ps, h, pg, pe, is_last = ent
            for t in range(2):
                i = 2 * pg + t
                if i < 4 * j:
                    ql = 0
                else:
                    ql = min(128 * (i - 4 * j), CH - 256)
                nc.tensor.matmul(
                    at_ps[:, ql:], lhsT=v[:, i, h, :],
                    rhs=pe[:, t * CH + ql:(t + 1) * CH],
                    start=(i == 0), stop=(is_last and t == 1),
                    skip_group_check=True)
            if is_last:
                closed.add(at_ps.tensor.name)

        def flush_ready():
            # emit norms only for heads whose accumulation group is closed
            # (emission order defines read/write semantics under Tile)
            while pending_norm and pending_norm[0][0].tensor.name in closed:
                at_ps, dst = pending_norm.pop(0)
                _norm_one(at_ps, dst)

        pend = []
        for h in range(HPC):
            mb, half = h // 2, h % 2
            row = slice(half * DK, (half + 1) * DK)
            at_ps = pp.tile([DK + 1, CH], F32, tag="at", bufs=3, name="at_ps")
            for g in range(nkb // 2):
                while carry_kv and carry_kv[0][0] <= (h, g):
                    carry_kv.popleft()[1]()
                i0 = 2 * g
                # Diagonal blocks are mostly masked: columns [0, qlo) of
                # k-block i are causally dead (q < k for the whole block),
                # so trim score/exp-mask/PV work to [qlo, CH).  fp32r
                # matmuls below N=256 run at 1/4 rate, so never trim
                # narrower than 256.
                def _qlo(i):
                    if i < 4 * j:
                        return 0
                    return min(128 * (i - 4 * j), CH - 256)

                sc = pp.tile([128, 2 * CH], F32, tag="sc", name="sc")
                pair_ql = _qlo(i0)  # uniform over the pair so the single
                # exp below reads only written PSUM
                for t in range(2):
                    i = i0 + t
                    nc.tensor.matmul(
                        sc[:, t * CH + pair_ql:(t + 1) * CH],
                        lhsT=kT[row, mb, i * 128:(i + 1) * 128],
                        rhs=qch[row, mb, pair_ql:], start=True, stop=True)
                e = epool.tile([128, 2 * CH], F32R, name="e", tag="e")
                sc_v = sc.rearrange("p (t c) -> p t c", t=2)[:, :, pair_ql:]
                e_v = e.rearrange("p (t c) -> p t c", t=2)[:, :, pair_ql:]
                nc.scalar.activation(out=e_v, in_=sc_v, func=EXP, scale=0.125)
                for t in range(2):
                    i = i0 + t
                    if i >= 4 * j:
                        # columns >= 128*(d+1) of the chunk are fully valid
                        # (q > every k in this block); columns < ql are
                        # never read by the trimmed pv.  Mask only between.
                        ql = _qlo(i)
                        hi = min(128 * (i - 4 * j + 1), CH)
                        nc.gpsimd.affine_select(
                            out=e[:, t * CH + ql:t * CH + hi],
                            in_=e[:, t * CH + ql:t * CH + hi],
                            compare_op=mybir.AluOpType.is_ge,
                            fill=0.0, base=j * CH - i * 128 + ql,
                            channel_multiplier=-1, pattern=[[1, hi - ql]])
                gstep += 1
                if int(gstep * nfill0 / ((1.0 + 0.2 * j) * steps)) >= npop + 1:
                    npop += 1
                    if carry_kv:
                        carry_kv.popleft()[1]()
                    elif fillers:
                        ent = fillers.popleft()
                        f = ent[-1]
                        if ent[0] == "next":
                            carry_next.append((ent[1], f))
                        else:
                            f()
                if len(pend) > 2:
                    emit_pv(pend.pop(0))
                flush_ready()
                pend.append((at_ps, h, g, e, g == nkb // 2 - 1))
            pending_norm.append((at_ps, ach[row, mb, :]))
        while pend:
            emit_pv(pend.pop(0))
        flush_ready()
        assert not pending_norm
        while carry_kv:
            carry_kv.popleft()[1]()
        while fillers:
            ent = fillers.popleft()
            if ent[0] == "next":
                carry_next.append((ent[1], ent[-1]))
            else:
                ent[-1]()
        carry_kv = carry_next
        carry_next = deque()
        prev = (j, ach)
        qch = qch_n

    jf, achf = prev
    for sb in range(4):
        sblk = jf * 4 + sb
        for n in range(2):
            ps = pp.tile([128, CH], F32, tag="sc", name="ps_of")
            for hp in range(4):
                nc.tensor.matmul(
                    ps, lhsT=achf[:, hp, sb * 128:(sb + 1) * 128],
                    rhs=wo[:, hp, n * CH:(n + 1) * CH],
                    start=(hp == 0), stop=(hp == 3))
            osb = opool.tile([128, CH], F32, name="osb", tag="osb")
            nc.vector.tensor_copy(osb, ps)
            nc.sync.dma_start(
                out=out[sblk * 128:(sblk + 1) * 128, n * CH:(n + 1) * CH],
                in_=osb)


def _split_excess_waits(nc, max_waits=1):
    # This walrus build rejects instructions carrying more than a couple of
    # sem waits ("Too many sync wait commands").  Engines execute their
    # stream in order, so excess waits can be moved onto nofuse nops placed
    # immediately before the instruction on the same engine.
    ctr = 0
    for blk in nc.m.functions[0].blocks:
        insts = blk.instructions
        out = []
        changed = False
        for inst in insts:
            si = inst.sync_info
            if si is not None and si.on_wait and len(si.on_wait) > max_waits:
                waits = list(si.on_wait)
                extra, keep = waits[:-max_waits], waits[-max_waits:]
                for gi in range(0, len(extra), max_waits):
                    ctr += 1
                    out.append(mybir.InstNoOp(
                        name=f"wsplit_{ctr}",
                        engine=inst.engine,
                        bass_nofuse=True,
                        sync_info=mybir.SyncInfo(
                            on_wait=extra[gi:gi + max_waits], on_update=[]),
                    ))
                inst.sync_info = mybir.SyncInfo(
                    on_wait=keep, on_update=si.on_update)
                changed = True
            out.append(inst)
        if changed:
            insts[:] = out


_CACHE = {}


def _get_nc(split=True):
    if "nc" in _CACHE:
        return _CACHE["nc"]
    tile.TileContext._drain_and_barrier = _drain_and_barrier_split
    nc = bass.Bass("TRN2", target_bir_lowering=False, debug=False)
    xT = nc.dram_tensor("xT", [D, S], F32R, kind="ExternalInput").ap()
    wqT = nc.dram_tensor("wqT", [D, HD], F32R, kind="ExternalInput").ap()
    wkT = nc.dram_tensor("wkT", [D, HD], F32R, kind="ExternalInput").ap()
    wvT = nc.dram_tensor("wvT", [D, HD], F32R, kind="ExternalInput").ap()
    woT = nc.dram_tensor("woT", [HD, D], F32R, kind="ExternalInput").ap()
    out = nc.dram_tensor("out", [S, D], F32, kind="ExternalOutput").ap()
    from contextlib import ExitStack
    with tile.TileContext(nc) as tc, ExitStack() as ctx:
        _build_kernel(ctx, tc, xT, wqT, wkT, wvT, woT, out)
    if split:
        _split_excess_waits(nc)
        _CACHE["nc"] = nc
    return nc


def make_in_maps(x, Wq, Wk, Wv, Wo):
    x = np.asarray(x, np.float32)
    Wq, Wk, Wv, Wo = (np.asarray(w, np.float32) for w in (Wq, Wk, Wv, Wo))
    in_maps = []
    for c in range(8):
        b, hh = c // 2, c % 2
        cols = slice(hh * HD, (hh + 1) * HD)
        in_maps.append({
            "xT": np.ascontiguousarray(x[b].T),
            "wqT": np.ascontiguousarray(Wq[cols, :].T),
            "wkT": np.ascontiguousarray(Wk[cols, :].T),
            "wvT": np.ascontiguousarray(Wv[cols, :].T),
            "woT": np.ascontiguousarray(Wo[:, cols].T),
        })
    return in_maps


def kernel(x, Wq, Wk, Wv, Wo, _trace=False, _trace_kwargs=None):
    nc = _get_nc()
    in_maps = make_in_maps(x, Wq, Wk, Wv, Wo)
    res = run_bass_kernel_spmd(
        nc, in_maps, core_ids=list(range(8)), trace=_trace,
        **(_trace_kwargs or {}))
    outs = [res.results[c]["out"] for c in range(8)]
    full = np.stack([outs[2 * b] + outs[2 * b + 1] for b in range(B)])
    if _trace:
        _CACHE["last_results"] = res
    return full.astype(np.float32)



# revision 16
# speedup vs baseline: 1.2736x; 1.2736x over previous
"""Causal multi-head self-attention on 8 Trainium2 NeuronCores.

Problem: x[4,2048,1024], Wq/Wk/Wv/Wo[1024,1024], H=16 heads, dk=64.
  q = x@Wq.T, k = x@Wk.T, v = x@Wv.T  (per-head causal softmax(q k^T/8) v) @ Wo.T

Sharding: core c handles batch b=c//2 and head-half hh=c%2 (8 heads).
Each core returns a partial transposed output outT[D,S] (its 512 attn
columns through the matching 512 rows of Wo.T); the host sums core
pairs and transposes.

Precision plan (rel-err budget 2e-2; lands ~5e-3):
  chunk 0 (q rows 0..511)   : bf16 everywhere (early rows have small
                              softmax support -> errors don't average).
  chunks 1-3 (rows 512..2047): fp8e4m3 with MatmulPerfMode.DoubleRow
                              (0.5 cycles/output column, 2x PE rate).
Host pre-quantizes and pre-packs x and all weights into the exact SBUF
tile layouts (including DoubleRow pair/slot packing and head
permutations), so the kernel DMAs everything linearly.

Kernel structure per chunk:
  scores^T [k,q] via DR matmuls (q/k stored [32-part, 2 dk-slot, s],
  4 heads per tile on PE quadrants 0/32/64/96 via explicit
  tile_position); exp on ACT (scale folds the 16*16 fp8 scaling,
  bias ln4 rescales e into fp8 range) writing fp8 e-tiles directly;
  causal masking = column trim + gpsimd affine_select zero-fill;
  PV reoriented as e.T@v -> at[q, 65] (65-column outputs; ones column
  of v gives the denominator; DR pairs 2 k-blocks per matmul);
  normalization = DVE reciprocal + per-partition tensor_scalar (the
  [q,dv] orientation makes the denominator a per-partition scalar);
  attn transposed back to [hd,q] with bf16 PE-transposes (2 heads per
  128x128 transpose) for the DoubleRow O-projection, which emits
  outT[d,q] tiles DMA'd to a transposed DRAM output.

exp on ACT is the bottleneck (~16.8M causal score elements per core at
1 elem/cycle/partition); PE work is interleaved into the exp shadow
via a filler deque (next chunk's projections, previous chunk's
transposes + O-projection).
"""

import numpy as np
import ml_dtypes

import concourse.bass as bass
import concourse.mybir as mybir
import concourse.tile as tile
from concourse.bass_utils import run_bass_kernel_spmd
from concourse.vector_clock import ScopedClock, VectorClock
from collections import deque

B, S, D, H, DK = 4, 2048, 1024, 16, 64
HPC = H // 2          # heads per core
CH = 512              # q-chunk width
NCH = S // CH         # 4
W = CH // 128         # q-windows per chunk (4)
F32 = mybir.dt.float32
F32R = mybir.dt.float32r
BF16 = mybir.dt.bfloat16
F8 = mybir.dt.float8e4
DR = mybir.MatmulPerfMode.DoubleRow
EXP = mybir.ActivationFunctionType.Exp
LN4 = float(np.log(4.0))
NPF8 = ml_dtypes.float8_e4m3
NPBF = ml_dtypes.bfloat16

XS, WS = 8.0, 256.0   # host scales: x8 = 8x, w8 = 256W
# fp8-projection psum = 2048*val -> q8/k8 stored 16*val, v8 stored 32*val
QCP = 2.0 ** -7
VCP = 2.0 ** -6
OCP = 2.0 ** -13      # O-proj psum (32*256=8192) -> out


def _drain_and_barrier_split(self, tick_clock, wait_clock):
    # The stock Tile tail drain attaches every outstanding sem wait to one
    # Drain instruction; this walrus build caps sync waits per instruction
    # and rejects it.  Put each wait on its own SP nop first, then drain
    # with no waits (SP has observed everything by then).
    gc = tick_clock.global_clock
    n = len(gc)
    for proc in range(n):
        t = gc[proc]
        if t == 0:
            continue
        vc = VectorClock([0] * n)
        vc.require_at_least(proc, t)
        nop = self.nc.sync.nop(nofuse=True)
        wait_clock.add_sem_waits(nop.ins, ScopedClock({None: vc}))
    self.nc.sync.drain()
    self.nc.all_engine_barrier()
    assert self.sems is not None
    popped = self.nc._tile_sem_poison_stack.pop()
    assert popped is self._sem_poison
    self.nc.clear_and_free_semaphores(list(self.sems.allocated().values()))
    self.nc.all_engine_barrier()


def _split_excess_waits(nc, max_waits=1):
    # This walrus build rejects instructions carrying more than a couple of
    # sem waits ("Too many sync wait commands").  Engines execute their
    # stream in order, so excess waits can be moved onto nofuse nops placed
    # immediately before the instruction on the same engine.
    ctr = 0
    for blk in nc.m.functions[0].blocks:
        insts = blk.instructions
        out = []
        changed = False
        for inst in insts:
            si = inst.sync_info
            if si is not None and si.on_wait and len(si.on_wait) > max_waits:
                waits = list(si.on_wait)
                extra, keep = waits[:-max_waits], waits[-max_waits:]
                for gi in range(0, len(extra), max_waits):
                    ctr += 1
                    out.append(mybir.InstNoOp(
                        name=f"wsplit_{ctr}",
                        engine=inst.engine,
                        bass_nofuse=True,
                        sync_info=mybir.SyncInfo(
                            on_wait=extra[gi:gi + max_waits], on_update=[]),
                    ))
                inst.sync_info = mybir.SyncInfo(
                    on_wait=keep, on_update=si.on_update)
                changed = True
            out.append(inst)
        if changed:
            insts[:] = out


def _build_kernel(ctx, tc, din, outT):
    nc = tc.nc

    wpool = ctx.enter_context(tc.tile_pool(name="weights", bufs=1))
    kvpool = ctx.enter_context(tc.tile_pool(name="kv", bufs=1))
    qpool = ctx.enter_context(tc.tile_pool(name="q", bufs=2))
    epool = ctx.enter_context(tc.tile_pool(name="exp", bufs=12))
    apool = ctx.enter_context(tc.tile_pool(name="attn", bufs=2))
    opool = ctx.enter_context(tc.tile_pool(name="osb", bufs=4))
    rpool = ctx.enter_context(tc.tile_pool(name="recip", bufs=2))
    # PSUM, 8 banks: sc 2x[128,2,512]f32 (4) + at 2x[128,4,128]f32 (2)
    #              + aux 2x one-bank tiles (2: fillers/transposes/O-proj)
    pp = ctx.enter_context(tc.tile_pool(name="pp", bufs=2, space="PSUM"))

    # ---- persistent tiles ----
    x8 = wpool.tile([128, 4, 2, S], F8, tag="x8")
    x16 = wpool.tile([128, 4, 2, CH], BF16, tag="x16")
    wq8 = wpool.tile([128, 4, 2, 2, 2, 128], F8, tag="wq8")
    wk8 = wpool.tile([128, 4, 2, 2, 2, 128], F8, tag="wk8")
    wv8 = wpool.tile([128, 4, 2, 512], F8, tag="wv8")
    wo8 = wpool.tile([128, 2, 2, D], F8, tag="wo8")
    wq16 = [wpool.tile([128, 4, 2, 128], BF16, tag=f"wq16_{mb}",
                       name=f"wq16_{mb}") for mb in range(4)]
    wk16 = [wpool.tile([128, 4, 2, 128], BF16, tag=f"wk16_{mb}",
                       name=f"wk16_{mb}") for mb in range(4)]
    wv16 = wpool.tile([128, 4, 2, 512], BF16, tag="wv16")
    wo16 = wpool.tile([128, 2, 2, D], BF16, tag="wo16")

    kT8 = [kvpool.tile([128, 2, S], F8, tag=f"kT8_{hg}", name=f"kT8_{hg}")
           for hg in range(2)]
    kT16 = kvpool.tile([128, 4, CH], BF16, tag="kT16")
    q16 = kvpool.tile([128, 4, CH], BF16, tag="q16")
    v8 = kvpool.tile([128, 8, 2, HPC, DK + 1], F8, tag="v8")
    v16 = kvpool.tile([128, 2, 2, HPC, DK + 1], BF16, tag="v16")
    attnT8 = kvpool.tile([128, 2, 2, S], F8, tag="attnT8")
    attnT16 = kvpool.tile([128, 2, 2, CH], BF16, tag="attnT16")

    ident = wpool.tile([128, 128], BF16, tag="ident")
    ln4ap = wpool.tile([128, 1], F32, tag="ln4")
    warm = wpool.tile([128, 128], F32R, tag="warm")

    # ---- init + warmup (under input DMAs) ----
    warm_f = wpool.tile([128, 128], F32, tag="warm_f")
    nc.gpsimd.memset(warm_f, 0.0)
    nc.gpsimd.tensor_copy(warm, warm_f)
    tmpf = wpool.tile([128, 128], F32, tag="tmpf")
    nc.vector.memset(tmpf, 1.0)
    nc.gpsimd.affine_select(out=tmpf, in_=tmpf,
                            compare_op=mybir.AluOpType.is_equal, fill=0.0,
                            base=0, channel_multiplier=-1, pattern=[[1, 128]])
    nc.vector.tensor_copy(ident, tmpf)
    nc.vector.memset(ln4ap, LN4)
    nc.vector.memset(v8[:, :, :, :, DK:DK + 1], 1.0)
    nc.vector.memset(v16[:, :, :, :, DK:DK + 1], 1.0)
    # preload the ACT exp table set early
    etw = rpool.tile([128, 16], F32, tag="etw", bufs=1)
    nc.scalar.activation(out=etw, in_=tmpf[:, 0:16], func=EXP, scale=1.0)
    # hold the PE clock-gate open / absorb the cold ramp while DMAs land
    wps = pp.tile([128, 2, CH], F32, tag="sc", name="wps")
    for r in range(24):
        nc.tensor.matmul(wps[:, r % 2, 0:128], lhsT=warm, rhs=warm,
                         start=True, stop=True)

    # ---- input DMAs (priority order on the sync queue) ----
    nc.sync.dma_start(out=wq16[0], in_=din["wq16"][:, 0])
    nc.sync.dma_start(out=wk16[0], in_=din["wk16"][:, 0])
    for pr in range(4):
        nc.sync.dma_start(out=x16[:, pr], in_=din["x16"][:, pr])
    nc.sync.dma_start(out=wv16, in_=din["wv16"])
    for mb in range(1, 4):
        nc.sync.dma_start(out=wq16[mb], in_=din["wq16"][:, mb])
        nc.sync.dma_start(out=wk16[mb], in_=din["wk16"][:, mb])
    nc.sync.dma_start(out=wq8, in_=din["wq8"])
    nc.sync.dma_start(out=wk8, in_=din["wk8"])
    nc.sync.dma_start(out=wv8, in_=din["wv8"])
    for half in range(2):
        cs = slice(half * S // 2, (half + 1) * S // 2)
        nc.sync.dma_start(out=x8[:, :, :, cs], in_=din["x8"][:, :, :, cs])
    nc.sync.dma_start(out=wo8, in_=din["wo8"])
    nc.sync.dma_start(out=wo16, in_=din["wo16"])

    # ---- chunk-0 bf16 projections ----
    # q16/kT16: baseline row layout (mb-block = 2 heads x 64 dk).
    def proj16q(mb, wtiles, dst, tag="aux"):
        def g():
            ps = pp.tile([128, CH], F32, tag=tag, bufs=None if tag == "sc"
                         else 2, name="pq16")
            for pr in range(4):
                for csl in range(2):
                    nc.tensor.matmul(
                        ps, lhsT=wtiles[mb][:, pr, csl, :],
                        rhs=x16[:, pr, csl, :],
                        start=(pr == 0 and csl == 0),
                        stop=(pr == 3 and csl == 1))
            nc.vector.tensor_copy(dst[:, mb, :], ps)
        return g

    def vproj16(sb):
        # v16 s-block sb (+ fp8 copy for later chunks)
        def f():
            ps = pp.tile([128, CH], F32, tag="aux", bufs=2, name="pv16")
            for pr in range(4):
                for csl in range(2):
                    nc.tensor.matmul(
                        ps, lhsT=x16[:, pr, csl, sb * 128:(sb + 1) * 128],
                        rhs=wv16[:, pr, csl, :],
                        start=(pr == 0 and csl == 0),
                        stop=(pr == 3 and csl == 1))
            pv = ps.rearrange("p (h d) -> p h d", h=HPC)
            nc.vector.tensor_copy(v16[:, sb // 2, sb % 2, :, 0:DK], pv)
            nc.vector.tensor_scalar(
                out=v8[:, sb // 2, sb % 2, :, 0:DK], in0=pv, scalar1=32.0,
                scalar2=None, op0=mybir.AluOpType.mult)
        return f

    def kproj8_c0(hg, sl):
        # fp8 K-projection of chunk-0 columns into kT8 (DR layout)
        def f():
            ps = pp.tile([128, CH], F32, tag="aux", bufs=2, name="pk8c0")
            for pr in range(4):
                nc.tensor.matmul(
                    ps, lhsT=wk8[:, pr, :, hg, sl, :],
                    rhs=x8[:, pr, :, 0:CH],
                    start=(pr == 0), stop=(pr == 3), perf_mode=DR)
            nc.vector.tensor_scalar(
                out=kT8[hg][:, sl, 0:CH], in0=ps, scalar1=QCP,
                scalar2=None, op0=mybir.AluOpType.mult)
        return f

    # ---- fp8 projections for chunk j>=1 ----
    def proj8(wsb, dstf, j, hg, sl):
        # q8/kT8 [4 heads x 32] dk-slot sl over chunk-j columns
        cs = slice(j * CH, (j + 1) * CH)
        def f():
            ps = pp.tile([128, CH], F32, tag="aux", bufs=2, name="p8")
            for pr in range(4):
                nc.tensor.matmul(
                    ps, lhsT=wsb[:, pr, :, hg, sl, :],
                    rhs=x8[:, pr, :, cs],
                    start=(pr == 0), stop=(pr == 3), perf_mode=DR)
            nc.vector.tensor_scalar(
                out=dstf(hg, sl, cs), in0=ps, scalar1=QCP, scalar2=None,
                op0=mybir.AluOpType.mult)
        return f

    def vproj8(j, sb):
        # v8 s-block 4j+sb
        def f():
            ps = pp.tile([128, CH], F32, tag="aux", bufs=2, name="pv8")
            blk = 4 * j + sb
            for pr in range(4):
                nc.tensor.matmul(
                    ps, lhsT=x8[:, pr, :, blk * 128:(blk + 1) * 128],
                    rhs=wv8[:, pr, :, :],
                    start=(pr == 0), stop=(pr == 3), perf_mode=DR)
            nc.vector.tensor_scalar(
                out=v8[:, blk // 2, blk % 2, :, 0:DK],
                in0=ps.rearrange("p (h d) -> p h d", h=HPC),
                scalar1=VCP, scalar2=None, op0=mybir.AluOpType.mult)
        return f

    # ---- filler machinery ----
    fillers = deque()    # (label, fn): deadline-ordered work
    bg = deque()         # no-deadline work (O-projections)
    done = set()

    def pop_filler(n=1):
        for _ in range(n):
            if fillers:
                label, f = fillers.popleft()
                f()
                done.add(label)
            elif bg:
                label, f = bg.popleft()
                f()
                done.add(label)
            else:
                return

    def need(label):
        while label not in done and fillers:
            lb, f = fillers.popleft()
            f()
            done.add(lb)

    # ---- per-chunk attention ----
    def emit_chunk(j, q8ch):
        """Attention for chunk j.  q8ch: [128, 2hg, 2sl, CH] fp8 tile for
        j>=1 (None for j=0, which reads q16/kT16/v16 in bf16)."""
        bf = j == 0
        at_tiles = []
        for h in range(HPC):
            if bf and h >= 2:
                need(f"q16_{h // 2}")
                need(f"k16_{h // 2}")
            at = pp.tile([128, W, 128], F32, tag="at", name=f"at{j}_{h}")
            npairs = 2 * (j + 1)
            es = []

            def qlo(i):
                return max(0, 128 * i - CH * j)

            def emit_pv_all():
                # One window at a time: a start=True marks the whole PSUM
                # bank pending-zero, so windows must be accumulated fully
                # before the next window's group begins.
                for w in range(W):
                    gmax = (4 * j + w) // 2
                    for g in range(gmax + 1):
                        e = es[g]
                        if bf:
                            for t in range(2):
                                i = 2 * g + t
                                if i > w:
                                    continue
                                nc.tensor.matmul(
                                    at[:, w, 0:DK + 1],
                                    lhsT=e[:, t, 128 * w:128 * (w + 1)],
                                    rhs=v16[:, g, t, h, :],
                                    start=(g == 0 and t == 0),
                                    stop=(i == w),
                                    skip_group_check=True)
                        else:
                            nc.tensor.matmul(
                                at[:, w, 0:DK + 1],
                                lhsT=e[:, :, 128 * w:128 * (w + 1)],
                                rhs=v8[:, g, :, h, :],
                                start=(g == 0), stop=(g == gmax),
                                perf_mode=DR, skip_group_check=True)

            for g in range(npairs):
                qlp = qlo(2 * g)
                sc = pp.tile([128, 2, CH], F32, tag="sc", name="sc")
                if bf:
                    mb, a = h // 2, h % 2
                    row = slice(a * DK, (a + 1) * DK)
                    for t in range(2):
                        i = 2 * g + t
                        ql = qlo(i)
                        nc.tensor.matmul(
                            sc[:, t, ql:],
                            lhsT=kT16[row, mb, 128 * i:128 * (i + 1)],
                            rhs=q16[row, mb, ql:],
                            start=True, stop=True)
                else:
                    hg, hq = h // 4, h % 4
                    rq = slice(32 * hq, 32 * hq + 32)
                    for t in range(2):
                        i = 2 * g + t
                        ql = qlo(i)
                        nc.tensor.matmul(
                            sc[:, t, ql:],
                            lhsT=kT8[hg][rq, :, 128 * i:128 * (i + 1)],
                            rhs=q8ch[rq, hg, :, ql:],
                            start=True, stop=True, perf_mode=DR,
                            tile_position=(32 * hq, 0))
                e = epool.tile([128, 2, CH], BF16 if bf else F8,
                               tag="e16" if bf else "e8", name=f"e{j}_{h}_{g}")
                nc.scalar.activation(
                    out=e[:, :, qlp:], in_=sc[:, :, qlp:], func=EXP,
                    scale=0.125 if bf else 2.0 ** -11, bias=ln4ap[:, 0:1])
                # causal masks on the diagonal blocks
                for t in range(2):
                    i = 2 * g + t
                    if i >= 4 * j:
                        hi = min(128 * (i - 4 * j) + 128, CH)
                        if hi > qlp:
                            nc.gpsimd.affine_select(
                                out=e[:, t, qlp:hi], in_=e[:, t, qlp:hi],
                                compare_op=mybir.AluOpType.is_ge, fill=0.0,
                                base=j * CH + qlp - i * 128,
                                channel_multiplier=-1,
                                pattern=[[1, hi - qlp]])
                es.append(e)
                pop_filler(2 if len(fillers) > 10 else 1)
            if bf:
                for sb in range(4):
                    need(f"v16_{sb}")
            emit_pv_all()
            at_tiles.append(at)
            # normalization: recip of the ones-column, per-partition scale
            rc = rpool.tile([128, W], F32, tag="rc", name=f"rc{j}_{h}")
            with nc.allow_low_precision(reason="softmax denominator recip"):
                nc.vector.reciprocal(out=rc, in_=at[:, :, DK])
            for w in range(W):
                nc.vector.tensor_scalar(
                    out=attn16[:, w, h, :], in0=at[:, w, 0:DK],
                    scalar1=rc[:, w:w + 1], scalar2=None,
                    op0=mybir.AluOpType.mult)
            if h % 2 == 1:
                # transpose this head pair as soon as both are normalized
                fillers.append((f"tr_{j}_{h // 2}",
                                transpose_fn(j, h // 2, attn16)))
        return at_tiles

    def transpose_fn(j, hp, attn16_t):
        bf = j == 0
        def f():
            tr = pp.tile([128, W, 256], BF16, tag="aux", bufs=2, name="tr")
            for w in range(W):
                nc.tensor.matmul(
                    tr[:, w, 0:128],
                    lhsT=attn16_t[:, w, 2 * hp:2 * hp + 2, :],
                    rhs=ident, is_transpose=True, start=True, stop=True)
            dst = attnT16 if bf else attnT8
            cs = slice(0, CH) if bf else slice(j * CH, (j + 1) * CH)
            dview = dst[:, hp // 2, hp % 2, cs].rearrange(
                "p (a b) -> p a b", a=W)
            nc.vector.tensor_copy(dview, tr[:, :, 0:128])
        return f

    def oproj_fn(j, n):
        bf = j == 0
        def f():
            ps = pp.tile([128, CH], F32, tag="aux", bufs=2, name="pjo")
            if bf:
                for pr in range(2):
                    for sl in range(2):
                        nc.tensor.matmul(
                            ps, lhsT=wo16[:, pr, sl, 128 * n:128 * (n + 1)],
                            rhs=attnT16[:, pr, sl, :],
                            start=(pr == 0 and sl == 0),
                            stop=(pr == 1 and sl == 1))
            else:
                cs = slice(j * CH, (j + 1) * CH)
                for pr in range(2):
                    nc.tensor.matmul(
                        ps, lhsT=wo8[:, pr, :, 128 * n:128 * (n + 1)],
                        rhs=attnT8[:, pr, :, cs],
                        start=(pr == 0), stop=(pr == 1), perf_mode=DR)
            osb = opool.tile([128, CH], F32, tag="osb", name="osb")
            if bf:
                nc.vector.tensor_copy(osb, ps)
            else:
                nc.vector.tensor_scalar(out=osb, in0=ps, scalar1=OCP,
                                        scalar2=None, op0=mybir.AluOpType.mult)
            eng = nc.sync if n % 2 else nc.gpsimd
            eng.dma_start(
                out=outT[128 * n:128 * (n + 1), j * CH:(j + 1) * CH], in_=osb)
        return f

    # ---- main schedule ----
    # chunk-0 critical projections first, rest as fillers
    proj16q(0, wq16, q16, tag="sc")()
    for kh in range(2):
        ps = pp.tile([128, 256], F32, tag="sc", name="pk0h")
        for pr in range(4):
            for csl in range(2):
                nc.tensor.matmul(
                    ps, lhsT=wk16[0][:, pr, csl, :],
                    rhs=x16[:, pr, csl, 256 * kh:256 * (kh + 1)],
                    start=(pr == 0 and csl == 0), stop=(pr == 3 and csl == 1))
        nc.vector.tensor_copy(kT16[:, 0, 256 * kh:256 * (kh + 1)], ps)
    fillers.append(("v16_0", vproj16(0)))
    fillers.append(("v16_1", vproj16(1)))
    fillers.append(("q16_1", proj16q(1, wq16, q16)))
    fillers.append(("k16_1", proj16q(1, wk16, kT16)))
    fillers.append(("v16_2", vproj16(2)))
    fillers.append(("v16_3", vproj16(3)))
    fillers.append(("q16_2", proj16q(2, wq16, q16)))
    fillers.append(("k16_2", proj16q(2, wk16, kT16)))
    fillers.append(("q16_3", proj16q(3, wq16, q16)))
    fillers.append(("k16_3", proj16q(3, wk16, kT16)))
    for hg in range(2):
        for sl in range(2):
            fillers.append((f"k8c0_{hg}{sl}", kproj8_c0(hg, sl)))

    def q8dst_fn(q8t):
        return lambda hg, sl, cs: q8t[:, hg, sl, :]

    def kdst_fn(hg, sl, cs):
        return kT8[hg][:, sl, cs]

    prev_j = None     # chunk awaiting its O-projection
    q8_next = None
    q8_cur = None
    for j in range(NCH):
        if j + 1 < NCH:
            q8_next = qpool.tile([128, 2, 2, CH], F8, tag="q8",
                                 name=f"q8_{j + 1}")
            for hg in range(2):
                for sl in range(2):
                    fillers.append((f"q8p_{j+1}_{hg}{sl}",
                                    proj8(wq8, q8dst_fn(q8_next), j + 1,
                                          hg, sl)))
                    fillers.append((f"k8p_{j+1}_{hg}{sl}",
                                    proj8(wk8, kdst_fn, j + 1, hg, sl)))
            for sb in range(4):
                fillers.append((f"v8p_{j+1}_{sb}", vproj8(j + 1, sb)))
        attn16 = apool.tile([128, W, HPC, DK], BF16, tag="attn16",
                            name=f"attn16_{j}")
        if prev_j is not None:
            for n in range(8):
                bg.append((f"o_{prev_j}_{n}", oproj_fn(prev_j, n)))
        if j >= 1:
            for hg in range(2):
                for sl in range(2):
                    need(f"q8p_{j}_{hg}{sl}")
                    need(f"k8p_{j}_{hg}{sl}")
        emit_chunk(j, q8ch=q8_cur if j >= 1 else None)
        prev_j = j
        q8_cur = q8_next
        # ensure all stragglers (e.g. v-projections) are in before next chunk
        if j + 1 < NCH:
            need(f"v8p_{j+1}_3")

    while fillers or bg:
        pop_filler()

    def oproj2_tail(n2):
        # last-chunk O-projection, 2 d-blocks per sc-tagged psum tile
        j = prev_j
        ps = pp.tile([128, 2, CH], F32, tag="sc", name="pot")
        cs = slice(j * CH, (j + 1) * CH)
        for t in range(2):
            n = 2 * n2 + t
            for pr in range(2):
                nc.tensor.matmul(
                    ps[:, t, :], lhsT=wo8[:, pr, :, 128 * n:128 * (n + 1)],
                    rhs=attnT8[:, pr, :, cs],
                    start=(pr == 0), stop=(pr == 1), perf_mode=DR)
        osb = opool.tile([128, 2, CH], F32, tag="osb2", name="osb2")
        nc.vector.tensor_scalar(out=osb, in0=ps, scalar1=OCP,
                                scalar2=None, op0=mybir.AluOpType.mult)
        for t in range(2):
            n = 2 * n2 + t
            eng = nc.sync if n % 2 else nc.gpsimd
            eng.dma_start(
                out=outT[128 * n:128 * (n + 1), j * CH:(j + 1) * CH],
                in_=osb[:, t, :])

    for n2 in range(4):
        oproj2_tail(n2)


_CACHE = {}


def _get_nc():
    if "nc" in _CACHE:
        return _CACHE["nc"]
    tile.TileContext._drain_and_barrier = _drain_and_barrier_split
    nc = bass.Bass("TRN2", target_bir_lowering=False, debug=False)
    din = {
        "x8": nc.dram_tensor("x8", [128, 4, 2, S], F8,
                             kind="ExternalInput").ap(),
        "x16": nc.dram_tensor("x16", [128, 4, 2, CH], BF16,
                              kind="ExternalInput").ap(),
        "wq8": nc.dram_tensor("wq8", [128, 4, 2, 2, 2, 128], F8,
                              kind="ExternalInput").ap(),
        "wk8": nc.dram_tensor("wk8", [128, 4, 2, 2, 2, 128], F8,
                              kind="ExternalInput").ap(),
        "wv8": nc.dram_tensor("wv8", [128, 4, 2, 512], F8,
                              kind="ExternalInput").ap(),
        "wo8": nc.dram_tensor("wo8", [128, 2, 2, D], F8,
                              kind="ExternalInput").ap(),
        "wq16": nc.dram_tensor("wq16", [128, 4, 4, 2, 128], BF16,
                               kind="ExternalInput").ap(),
        "wk16": nc.dram_tensor("wk16", [128, 4, 4, 2, 128], BF16,
                               kind="ExternalInput").ap(),
        "wv16": nc.dram_tensor("wv16", [128, 4, 2, 512], BF16,
                               kind="ExternalInput").ap(),
        "wo16": nc.dram_tensor("wo16", [128, 2, 2, D], BF16,
                               kind="ExternalInput").ap(),
    }
    outT = nc.dram_tensor("outT", [D, S], F32, kind="ExternalOutput").ap()
    from contextlib import ExitStack
    with tile.TileContext(nc) as tc, ExitStack() as ctx:
        _build_kernel(ctx, tc, din, outT)
    _split_excess_waits(nc)
    _CACHE["nc"] = nc
    return nc


def make_in_maps(x, Wq, Wk, Wv, Wo):
    x = np.asarray(x, np.float32)
    Wq, Wk, Wv, Wo = (np.asarray(w, np.float32) for w in (Wq, Wk, Wv, Wo))
    xb8, xb16 = [], []
    for b in range(B):
        xT = np.ascontiguousarray(x[b].T)                      # [D, S]
        x4 = xT.reshape(4, 2, 128, S).transpose(2, 0, 1, 3)    # [128,4,2,S]
        xb8.append(np.ascontiguousarray((x4 * XS)).astype(NPF8))
        xb16.append(np.ascontiguousarray(x4[:, :, :, :CH]).astype(NPBF))

    def pack_dr_qk(Wm, hh):
        # [128p, 4pr, 2csl, 2hg, 2qsl, 128(h*32+dk)]
        Wc = Wm[512 * hh:512 * hh + 512, :]
        a = Wc.reshape(2, 4, 2, 32, 4, 2, 128)   # hg,h,qsl,dk,pr,csl,p
        a = a.transpose(6, 4, 5, 0, 2, 1, 3)     # p,pr,csl,hg,qsl,h,dk
        return np.ascontiguousarray(a.reshape(128, 4, 2, 2, 2, 128))

    def pack_16_qk(Wm, hh):
        # [128p, 4mb, 4pr, 2csl, 128(a*64+dk)]
        Wc = Wm[512 * hh:512 * hh + 512, :]
        a = Wc.reshape(4, 2, 64, 4, 2, 128)      # mb,a,dk,pr,csl,p
        a = a.transpose(5, 0, 3, 4, 1, 2)        # p,mb,pr,csl,a,dk
        return np.ascontiguousarray(a.reshape(128, 4, 4, 2, 128))

    def pack_v(Wm, hh):
        # [128p, 4pr, 2csl, 512(h*64+dv)]
        Wc = Wm[512 * hh:512 * hh + 512, :]
        a = Wc.reshape(8, 64, 4, 2, 128)         # h,dv,pr,csl,p
        a = a.transpose(4, 2, 3, 0, 1)           # p,pr,csl,h,dv
        return np.ascontiguousarray(a.reshape(128, 4, 2, 512))

    def pack_o(Wm, hh):
        # [128p, 2pr, 2sl, 1024n]
        Wc = Wm[:, 512 * hh:512 * hh + 512].T    # [512 hd, 1024 n]
        a = Wc.reshape(2, 2, 128, D)             # pr,sl,p,n
        return np.ascontiguousarray(a.transpose(2, 0, 1, 3))

    packs = []
    for hh in range(2):
        packs.append({
            "wq8": (pack_dr_qk(Wq, hh) * WS).astype(NPF8),
            "wk8": (pack_dr_qk(Wk, hh) * WS).astype(NPF8),
            "wv8": (pack_v(Wv, hh) * WS).astype(NPF8),
            "wo8": (pack_o(Wo, hh) * WS).astype(NPF8),
            "wq16": pack_16_qk(Wq, hh).astype(NPBF),
            "wk16": pack_16_qk(Wk, hh).astype(NPBF),
            "wv16": pack_v(Wv, hh).astype(NPBF),
            "wo16": pack_o(Wo, hh).astype(NPBF),
        })

    in_maps = []
    for c in range(8):
        b, hh = c // 2, c % 2
        m = {"x8": xb8[b], "x16": xb16[b]}
        m.update(packs[hh])
        in_maps.append(m)
    return in_maps


def kernel(x, Wq, Wk, Wv, Wo, _trace=False, _trace_kwargs=None):
    nc = _get_nc()
    in_maps = make_in_maps(x, Wq, Wk, Wv, Wo)
    res = run_bass_kernel_spmd(
        nc, in_maps, core_ids=list(range(8)), trace=_trace,
        **(_trace_kwargs or {}))
    outs = [res.results[c]["outT"] for c in range(8)]
    full = np.stack([(outs[2 * b] + outs[2 * b + 1]).T for b in range(B)])
    if _trace:
        _CACHE["last_results"] = res
    return full.astype(np.float32)


# revision 45
# speedup vs baseline: 1.5525x; 1.2190x over previous
"""Causal multi-head self-attention on 8 Trainium2 NeuronCores.

Problem: x[4,2048,1024], Wq/Wk/Wv/Wo[1024,1024], H=16 heads, dk=64.
  q = x@Wq.T, k = x@Wk.T, v = x@Wv.T  (per-head causal softmax(q k^T/8) v) @ Wo.T

Sharding: core c handles batch b=c//2 and head-half hh=c%2 (8 heads).
Each core returns a partial transposed output outT[D,S] (its 512 attn
columns through the matching 512 rows of Wo.T); the host sums core
pairs and transposes.

Precision plan (rel-err budget 2e-2; lands ~5e-3):
  chunk 0 (q rows 0..511)   : bf16 everywhere (early rows have small
                              softmax support -> errors don't average).
  chunks 1-3 (rows 512..2047): fp8e4m3 with MatmulPerfMode.DoubleRow
                              (0.5 cycles/output column, 2x PE rate).
Host pre-quantizes and pre-packs x and all weights into the exact SBUF
tile layouts (including DoubleRow pair/slot packing and head
permutations), so the kernel DMAs everything linearly.

Kernel structure per chunk:
  scores^T [k,q] via DR matmuls (q/k stored [32-part, 2 dk-slot, s],
  4 heads per tile on PE quadrants 0/32/64/96 via explicit
  tile_position); exp on ACT (scale folds the 16*16 fp8 scaling,
  bias ln4 rescales e into fp8 range) writing fp8 e-tiles directly;
  causal masking = column trim + gpsimd affine_select zero-fill;
  PV reoriented as e.T@v -> at[q, 65] (65-column outputs; ones column
  of v gives the denominator; DR pairs 2 k-blocks per matmul);
  normalization = DVE reciprocal + per-partition tensor_scalar (the
  [q,dv] orientation makes the denominator a per-partition scalar);
  attn transposed back to [hd,q] with bf16 PE-transposes (2 heads per
  128x128 transpose) for the DoubleRow O-projection, which emits
  outT[d,q] tiles DMA'd to a transposed DRAM output.

exp on ACT is the bottleneck (~16.8M causal score elements per core at
1 elem/cycle/partition); PE work is interleaved into the exp shadow
via a filler deque (next chunk's projections, previous chunk's
transposes + O-projection).
"""

import numpy as np
import ml_dtypes

import concourse.bass as bass
import concourse.mybir as mybir
import concourse.tile as tile
from concourse.bass_utils import run_bass_kernel_spmd
from concourse.vector_clock import ScopedClock, VectorClock
from collections import deque

B, S, D, H, DK = 4, 2048, 1024, 16, 64
HPC = H // 2          # heads per core
CH = 512              # q-chunk width
NCH = S // CH         # 4
W = CH // 128         # q-windows per chunk (4)
F32 = mybir.dt.float32
I32 = mybir.dt.int32
F32R = mybir.dt.float32r
BF16 = mybir.dt.bfloat16
F8 = mybir.dt.float8e4
DR = mybir.MatmulPerfMode.DoubleRow
EXP = mybir.ActivationFunctionType.Exp
LN4 = float(np.log(4.0))
NPF8 = ml_dtypes.float8_e4m3
NPBF = ml_dtypes.bfloat16

XS, WS = 8.0, 256.0   # host scales: x8 = 8x, w8 = 256W
# Schraudolph exp2 bit-trick constants for offloaded exps (DVE+Pool):
# i32 = round(sc * 2^-11 * log2e * 2^23 + (129 - cadj) * 2^23); bitcast f32
# gives ~4*exp(sc/2048) with ~3% piecewise-linear error.
EXA = float(np.float32(1.4426950408889634 * (1 << 23) * 2.0 ** -11))
EXB = float(np.float32((129.0 - 0.044) * (1 << 23)))
# fp8-projection psum = 2048*val -> q8/k8 stored 16*val, v8 stored 32*val
QCP = 2.0 ** -7
VCP = 2.0 ** -6
OCP = 2.0 ** -13      # O-proj psum (32*256=8192) -> out


def _drain_and_barrier_split(self, tick_clock, wait_clock):
    # The stock Tile tail drain attaches every outstanding sem wait to one
    # Drain instruction; this walrus build caps sync waits per instruction
    # and rejects it.  Put each wait on its own SP nop first, then drain
    # with no waits (SP has observed everything by then).
    gc = tick_clock.global_clock
    n = len(gc)
    for proc in range(n):
        t = gc[proc]
        if t == 0:
            continue
        vc = VectorClock([0] * n)
        vc.require_at_least(proc, t)
        nop = self.nc.sync.nop(nofuse=True)
        wait_clock.add_sem_waits(nop.ins, ScopedClock({None: vc}))
    self.nc.sync.drain()
    self.nc.all_engine_barrier()
    assert self.sems is not None
    popped = self.nc._tile_sem_poison_stack.pop()
    assert popped is self._sem_poison
    self.nc.clear_and_free_semaphores(list(self.sems.allocated().values()))
    self.nc.all_engine_barrier()


def _split_excess_waits(nc, max_waits=1):
    # This walrus build rejects instructions carrying more than a couple of
    # sem waits ("Too many sync wait commands").  Engines execute their
    # stream in order, so excess waits can be moved onto nofuse nops placed
    # immediately before the instruction on the same engine.
    ctr = 0
    for blk in nc.m.functions[0].blocks:
        insts = blk.instructions
        out = []
        changed = False
        for inst in insts:
            si = inst.sync_info
            if si is not None and si.on_wait and len(si.on_wait) > max_waits:
                waits = list(si.on_wait)
                extra, keep = waits[:-max_waits], waits[-max_waits:]
                for gi in range(0, len(extra), max_waits):
                    ctr += 1
                    out.append(mybir.InstNoOp(
                        name=f"wsplit_{ctr}",
                        engine=inst.engine,
                        bass_nofuse=True,
                        sync_info=mybir.SyncInfo(
                            on_wait=extra[gi:gi + max_waits], on_update=[]),
                    ))
                inst.sync_info = mybir.SyncInfo(
                    on_wait=keep, on_update=si.on_update)
                changed = True
            out.append(inst)
        if changed:
            insts[:] = out


def _build_kernel(ctx, tc, din, outT, outT16):
    nc = tc.nc

    wpool = ctx.enter_context(tc.tile_pool(name="weights", bufs=1))
    kvpool = ctx.enter_context(tc.tile_pool(name="kv", bufs=1))
    qpool = ctx.enter_context(tc.tile_pool(name="q", bufs=2))
    epool = ctx.enter_context(tc.tile_pool(name="exp", bufs=16))
    apool = ctx.enter_context(tc.tile_pool(name="attn", bufs=2))
    opool = ctx.enter_context(tc.tile_pool(name="osb", bufs=4))
    rpool = ctx.enter_context(tc.tile_pool(name="recip", bufs=2))
    # PSUM, 8 banks: sc 2x[128,2,512]f32 (4: ACT-consumed score ring)
    #   + sco 2x[128,512]f32 (2: DVE-consumed offloaded-score ring)
    #   + at 1x[128,4,128]f32 (1: PV accumulator; single-buffered thanks to
    #     the deferred head-tail) + aux 1x (1: fillers/transposes/O-proj)
    pp = ctx.enter_context(tc.tile_pool(name="pp", bufs=2, space="PSUM"))

    # ---- persistent tiles ----
    x8 = wpool.tile([128, 4, 2, S], F8, tag="x8")
    x16 = wpool.tile([128, 4, 2, 256], BF16, tag="x16")
    wq8 = wpool.tile([128, 4, 2, 2, 2, 128], F8, tag="wq8")
    wk8 = wpool.tile([128, 4, 2, 2, 2, 128], F8, tag="wk8")
    wv8 = wpool.tile([128, 4, 2, 512], F8, tag="wv8")
    wo8 = wpool.tile([128, 2, 2, D], F8, tag="wo8")
    wq16 = [wpool.tile([128, 4, 2, 128], BF16, tag=f"wq16_{mb}",
                       name=f"wq16_{mb}") for mb in range(4)]
    wk16 = [wpool.tile([128, 4, 2, 128], BF16, tag=f"wk16_{mb}",
                       name=f"wk16_{mb}") for mb in range(4)]
    wv16 = wpool.tile([128, 4, 2, 512], BF16, tag="wv16")
    wo16 = wpool.tile([128, 2, 2, D], BF16, tag="wo16")

    kT8 = [kvpool.tile([128, 2, S], F8, tag=f"kT8_{hg}", name=f"kT8_{hg}")
           for hg in range(2)]
    kT16 = kvpool.tile([128, 4, 256], BF16, tag="kT16")
    q16 = kvpool.tile([128, 4, 256], BF16, tag="q16")
    v8 = kvpool.tile([128, 8, 2, HPC, DK + 1], F8, tag="v8")
    v16 = kvpool.tile([128, 2, 2, HPC, DK + 1], BF16, tag="v16")
    attnT8 = kvpool.tile([128, 2, 2, S], F8, tag="attnT8")
    attnT16 = kvpool.tile([128, 2, 2, CH], BF16, tag="attnT16")

    ident = wpool.tile([128, 128], BF16, tag="ident")
    ln4ap = wpool.tile([128, 1], F32, tag="ln4")
    warm = wpool.tile([128, 128], F32R, tag="warm")

    # ---- input DMAs (priority order on the sync queue) ----
    nc.sync.dma_start(out=wk8, in_=din["wk8"])
    nc.sync.dma_start(out=wq8, in_=din["wq8"])
    csq = slice(0, CH)
    nc.sync.dma_start(out=x8[:, :, :, csq], in_=din["x8"][:, :, :, csq])
    for pr in range(4):
        nc.sync.dma_start(out=x16[:, pr], in_=din["x16"][:, pr])
    nc.sync.dma_start(out=wq16[0], in_=din["wq16"][:, 0])
    nc.sync.dma_start(out=wk16[0], in_=din["wk16"][:, 0])
    nc.sync.dma_start(out=wv16, in_=din["wv16"])
    nc.sync.dma_start(out=wv8, in_=din["wv8"])
    csb = slice(CH, 2 * CH)
    nc.sync.dma_start(out=x8[:, :, :, csb], in_=din["x8"][:, :, :, csb])
    for mb in range(1, 4):
        nc.sync.dma_start(out=wq16[mb], in_=din["wq16"][:, mb])
        nc.sync.dma_start(out=wk16[mb], in_=din["wk16"][:, mb])
    csr = slice(2 * CH, S)
    nc.sync.dma_start(out=x8[:, :, :, csr], in_=din["x8"][:, :, :, csr])
    nc.sync.dma_start(out=wo8, in_=din["wo8"])
    nc.sync.dma_start(out=wo16, in_=din["wo16"])

    # ---- init + warmup (under input DMAs) ----
    warm_f = wpool.tile([128, 128], F32, tag="warm_f")
    nc.gpsimd.memset(warm_f, 0.0)
    nc.gpsimd.tensor_copy(warm, warm_f)
    tmpf = wpool.tile([128, 128], F32, tag="tmpf")
    nc.vector.memset(tmpf, 1.0)
    nc.gpsimd.affine_select(out=tmpf, in_=tmpf,
                            compare_op=mybir.AluOpType.is_equal, fill=0.0,
                            base=0, channel_multiplier=-1, pattern=[[1, 128]])
    nc.vector.tensor_copy(ident, tmpf)
    nc.vector.memset(ln4ap, LN4)
    nc.vector.memset(v8[:, :, :, :, DK:DK + 1], 1.0)
    nc.vector.memset(v16[:, :, :, :, DK:DK + 1], 1.0)
    # preload the ACT exp table set early
    etw = rpool.tile([128, 16], F32, tag="etw", bufs=1)
    nc.scalar.activation(out=etw, in_=tmpf[:, 0:16], func=EXP, scale=1.0)
    # hold the PE clock-gate open / absorb the cold ramp while DMAs land
    wps = pp.tile([128, 2, CH], F32, tag="sc", name="wps")
    for r in range(26):
        nc.tensor.matmul(wps[:, r % 2, 0:128], lhsT=warm, rhs=warm,
                         start=True, stop=True)

    # ---- chunk-0 bf16 projections ----
    # q16/kT16: baseline row layout (mb-block = 2 heads x 64 dk).
    def proj16q(mb, wtiles, dst, tag="sco"):
        def g():
            ps = pp.tile([128, 256], F32, tag=tag, bufs=None if tag == "sc"
                         else 2, name="pq16")
            for pr in range(4):
                for csl in range(2):
                    nc.tensor.matmul(
                        ps, lhsT=wtiles[mb][:, pr, csl, :],
                        rhs=x16[:, pr, csl, :],
                        start=(pr == 0 and csl == 0),
                        stop=(pr == 3 and csl == 1))
            nc.vector.tensor_copy(dst[:, mb, :], ps)
        return g

    def vproj16(sb):
        # v16 s-block sb (+ fp8 copy for later chunks)
        def f():
            ps = pp.tile([128, CH], F32, tag="sco", bufs=2, name="pv16")
            for pr in range(4):
                for csl in range(2):
                    nc.tensor.matmul(
                        ps, lhsT=x16[:, pr, csl, sb * 128:(sb + 1) * 128],
                        rhs=wv16[:, pr, csl, :],
                        start=(pr == 0 and csl == 0),
                        stop=(pr == 3 and csl == 1))
            pv = ps.rearrange("p (h d) -> p h d", h=HPC)
            nc.vector.tensor_copy(v16[:, sb // 2, sb % 2, :, 0:DK], pv)
            nc.vector.tensor_scalar(
                out=v8[:, sb // 2, sb % 2, :, 0:DK], in0=pv, scalar1=32.0,
                scalar2=None, op0=mybir.AluOpType.mult)
        return f

    def kproj8_c0(hg, sl):
        # fp8 K-projection of chunk-0 columns into kT8 (DR layout)
        def f():
            ps = pp.tile([128, CH], F32, tag="sco", bufs=2, name="pk8c0")
            for pr in range(4):
                nc.tensor.matmul(
                    ps, lhsT=wk8[:, pr, :, hg, sl, :],
                    rhs=x8[:, pr, :, 0:CH],
                    start=(pr == 0), stop=(pr == 3), perf_mode=DR)
            nc.vector.tensor_scalar(
                out=kT8[hg][:, sl, 0:CH], in0=ps, scalar1=QCP,
                scalar2=None, op0=mybir.AluOpType.mult)
        return f

    # ---- fp8 projections for chunk j>=1 ----
    def proj8(wsb, dstf, j, hg, sl, cs=None, tag="sco"):
        # q8/kT8 [4 heads x 32] dk-slot sl over chunk-j columns
        if cs is None:
            cs = slice(j * CH, (j + 1) * CH)
        def f():
            ps = pp.tile([128, cs.stop - cs.start], F32, tag=tag,
                         bufs=2 if tag == "sco" else 1, name="p8")
            for pr in range(4):
                nc.tensor.matmul(
                    ps, lhsT=wsb[:, pr, :, hg, sl, :],
                    rhs=x8[:, pr, :, cs],
                    start=(pr == 0), stop=(pr == 3), perf_mode=DR)
            nc.vector.tensor_scalar(
                out=dstf(hg, sl, cs), in0=ps, scalar1=QCP, scalar2=None,
                op0=mybir.AluOpType.mult)
        return f

    def vproj8(j, sb, tag="sco"):
        # v8 s-block 4j+sb
        def f():
            ps = pp.tile([128, CH], F32, tag=tag,
                         bufs=2 if tag == "sco" else 1, name="pv8")
            blk = 4 * j + sb
            for pr in range(4):
                nc.tensor.matmul(
                    ps, lhsT=x8[:, pr, :, blk * 128:(blk + 1) * 128],
                    rhs=wv8[:, pr, :, :],
                    start=(pr == 0), stop=(pr == 3), perf_mode=DR)
            nc.vector.tensor_scalar(
                out=v8[:, blk // 2, blk % 2, :, 0:DK],
                in0=ps.rearrange("p (h d) -> p h d", h=HPC),
                scalar1=VCP, scalar2=None, op0=mybir.AluOpType.mult)
        return f

    # ---- filler machinery ----
    fillers = deque()    # (label, fn): deadline-ordered work
    bg = deque()         # no-deadline work (O-projections)
    done = set()

    def pop_filler(n=1):
        for _ in range(n):
            if fillers:
                label, f = fillers.popleft()
                f()
                done.add(label)
            elif bg:
                label, f = bg.popleft()
                f()
                done.add(label)
            else:
                return

    def need(label):
        while label not in done and fillers:
            lb, f = fillers.popleft()
            f()
            done.add(lb)

    # ---- per-chunk attention ----
    def emit_chunk(j, q8ch):
        """Attention for chunk j.  q8ch: [128, 2hg, 2sl, CH] fp8 q tile
        (chunk 0 only holds columns 256:512; its windows 0-1 run on the
        bf16 q16/kT16/v16 path)."""
        bf = j == 0
        for h in range(HPC):
            if bf and h >= 2:
                need(f"q16_{h // 2}")
                need(f"k16_{h // 2}")
            at = pp.tile([128, W, 128], F32, tag="at", bufs=1,
                         name=f"at{j}_{h}")
            npairs = 2 * (j + 1)
            es = [None] * npairs

            def qlo(i):
                return max(0, 128 * i - CH * j)

            def emit_pv_all(h=h, at=at, es=es):
                # One window at a time: a start=True marks the whole PSUM
                # bank pending-zero, so windows must be accumulated fully
                # before the next window's group begins.
                for w in range(W):
                    gmax = (4 * j + w) // 2
                    if bf and w < 2:
                        e16 = es[0][0]
                        for t in range(w + 1):
                            nc.tensor.matmul(
                                at[:, w, 0:DK + 1],
                                lhsT=e16[:, t, 128 * w:128 * (w + 1)],
                                rhs=v16[:, 0, t, h, :],
                                start=(t == 0), stop=(t == w),
                                skip_group_check=True)
                        continue
                    for g in range(gmax + 1):
                        e8 = es[g][1] if bf else es[g]
                        nc.tensor.matmul(
                            at[:, w, 0:DK + 1],
                            lhsT=e8[:, :, 128 * w:128 * (w + 1)],
                            rhs=v8[:, g, :, h, :],
                            start=(g == 0), stop=(g == gmax),
                            perf_mode=DR, skip_group_check=True)

            gorder = list(range(npairs))
            if j == NCH - 1 and h == HPC - 1:
                # last head: diagonal (masked) pairs first so the final
                # PV burst doesn't wait on late pool-side masks
                gorder = gorder[-2:] + gorder[:-2]
            for g in gorder:
                qlp = qlo(2 * g)
                offl = ((j == 2 and g == 1)
                        or (j == 3 and g in (1, 3, 5)
                            and not (h == HPC - 1 and g == 5)))
                if offl:
                    sco = [pp.tile([128, CH], F32, tag="sco", bufs=2,
                                   name=f"sco{t}") for t in range(2)]
                    sc = None
                else:
                    sc = pp.tile([128, 2, CH], F32, tag="sc", name="sc")
                # fp8 scores for columns [max(ql,256*bf):512)
                hg, hq = h // 4, h % 4
                if bf and g == 0:
                    for sl in range(2):
                        need(f"k8c0_{hg}{sl}")
                        need(f"q8c0_{hg}{sl}")
                rq = slice(32 * hq, 32 * hq + 32)
                for t in range(2):
                    i = 2 * g + t
                    ql = max(qlo(i), 256) if bf else qlo(i)
                    nc.tensor.matmul(
                        sco[t][:, ql:] if offl else sc[:, t, ql:],
                        lhsT=kT8[hg][rq, :, 128 * i:128 * (i + 1)],
                        rhs=q8ch[rq, hg, :, ql:],
                        start=True, stop=True, perf_mode=DR,
                        tile_position=(32 * hq, 0))
                if bf:
                    e8 = epool.tile([128, 2, CH], F8, tag="e8",
                                    name=f"e8_{j}_{h}_{g}")
                    q8l = max(qlp, 256)
                    nc.scalar.activation(
                        out=e8[:, :, q8l:], in_=sc[:, :, q8l:], func=EXP,
                        scale=2.0 ** -11, bias=ln4ap[:, 0:1])
                    for t in range(2):
                        i = 2 * g + t
                        hi8 = min(128 * i + 128, CH)
                        if hi8 > q8l:
                            nc.gpsimd.affine_select(
                                out=e8[:, t, q8l:hi8],
                                in_=e8[:, t, q8l:hi8],
                                compare_op=mybir.AluOpType.is_ge,
                                fill=0.0, base=q8l - i * 128,
                                channel_multiplier=-1,
                                pattern=[[1, hi8 - q8l]])
                    es[g] = (None, e8)
                else:
                    e = epool.tile([128, 2, CH], F8, tag="e8",
                                   name=f"e{j}_{h}_{g}")
                    if offl:
                        # offload to DVE (exp2 bit-trick) + Pool (fp8 cast)
                        i32 = epool.tile([128, 2, CH], I32, tag="i32",
                                         bufs=4, name=f"i{j}_{h}_{g}")
                        for t in range(2):
                            nc.vector.tensor_scalar(
                                out=i32[:, t, :], in0=sco[t], scalar1=EXA,
                                scalar2=EXB, op0=mybir.AluOpType.mult,
                                op1=mybir.AluOpType.add)
                        nc.gpsimd.tensor_copy(e, i32.bitcast(F32))
                    else:
                        nc.scalar.activation(
                            out=e[:, :, qlp:], in_=sc[:, :, qlp:], func=EXP,
                            scale=2.0 ** -11, bias=ln4ap[:, 0:1])
                    for t in range(2):
                        i = 2 * g + t
                        if i >= 4 * j:
                            hi = min(128 * (i - 4 * j) + 128, CH)
                            if hi > qlp:
                                nc.gpsimd.affine_select(
                                    out=e[:, t, qlp:hi], in_=e[:, t, qlp:hi],
                                    compare_op=mybir.AluOpType.is_ge,
                                    fill=0.0, base=j * CH + qlp - i * 128,
                                    channel_multiplier=-1,
                                    pattern=[[1, hi - qlp]])
                    es[g] = e
                if g == gorder[1] and pend_tail[0] is not None:
                    pend_tail[0]()
                    pend_tail[0] = None
                if j < 2:
                    pop_filler(2 if len(fillers) > 10 else 1)
                elif g not in gorder[:2]:
                    pop_filler(1)

            if bf:
                # bf16 scores + exp for columns [0:256) (windows 0-1),
                # emitted after the head's fp8 stream so the late-arriving
                # x16/wq16 DMAs never stall the fp8 pipeline
                need("q16_0")
                need("k16_0")
                mb, a = h // 2, h % 2
                row = slice(a * DK, (a + 1) * DK)
                sc16 = pp.tile([128, 2, 256], F32, tag="sco", bufs=2,
                               name="sc16")
                for t in range(2):
                    ql = qlo(t)
                    nc.tensor.matmul(
                        sc16[:, t, ql:],
                        lhsT=kT16[row, mb, 128 * t:128 * (t + 1)],
                        rhs=q16[row, mb, ql:],
                        start=True, stop=True)
                e16 = epool.tile([128, 2, 256], BF16, tag="e16",
                                 name=f"e16_{h}")
                nc.scalar.activation(
                    out=e16, in_=sc16, func=EXP,
                    scale=0.125, bias=ln4ap[:, 0:1])
                for t in range(2):
                    hi16 = min(128 * t + 128, 256)
                    nc.gpsimd.affine_select(
                        out=e16[:, t, 0:hi16], in_=e16[:, t, 0:hi16],
                        compare_op=mybir.AluOpType.is_ge,
                        fill=0.0, base=-t * 128,
                        channel_multiplier=-1, pattern=[[1, hi16]])
                es[0] = (e16, es[0][1])

            def head_tail(h=h, at=at, es=es, attn16_t=attn16,
                          emit_pv_all=emit_pv_all):
                if bf:
                    need("v16_0")
                    need("v16_1")
                    need("v8p_0_2")
                    need("v8p_0_3")
                emit_pv_all()
                # normalization: recip of the ones-column, then one
                # broadcast multiply over all four windows
                rc = rpool.tile([128, W, 1], F32, tag="rc",
                                name=f"rc{j}_{h}")
                with nc.allow_low_precision(
                        reason="softmax denominator recip"):
                    nc.vector.reciprocal(out=rc[:, :, 0], in_=at[:, :, DK])
                if bf:
                    # windows 2-3 carry the fp8 32x scale; fold 1/32 into rc
                    nc.vector.tensor_scalar(
                        out=rc[:, 2:4, 0], in0=rc[:, 2:4, 0],
                        scalar1=2.0 ** -5, scalar2=None,
                        op0=mybir.AluOpType.mult)
                nc.vector.tensor_tensor(
                    out=attn16_t[:, :, h, :], in0=at[:, :, 0:DK],
                    in1=rc.to_broadcast([128, W, DK]),
                    op=mybir.AluOpType.mult)
                if h % 2 == 1:
                    # transpose this head pair once both are normalized
                    fillers.append((f"tr_{j}_{h // 2}",
                                    transpose_fn(j, h // 2, attn16_t)))
            if pend_tail[0] is not None:
                pend_tail[0]()
            pend_tail[0] = head_tail

    def transpose_fn(j, hp, attn16_t):
        bf = j == 0
        def f():
            tr = pp.tile([128, W, 256], BF16, tag="aux", bufs=1, name="tr")
            for w in range(W):
                nc.tensor.matmul(
                    tr[:, w, 0:128],
                    lhsT=attn16_t[:, w, 2 * hp:2 * hp + 2, :],
                    rhs=ident, is_transpose=True, start=True, stop=True)
            dst = attnT16 if bf else attnT8
            cs = slice(0, CH) if bf else slice(j * CH, (j + 1) * CH)
            dview = dst[:, hp // 2, hp % 2, cs].rearrange(
                "p (a b) -> p a b", a=W)
            nc.vector.tensor_copy(dview, tr[:, :, 0:128])
        return f

    def oproj_fn(j, n):
        # O-projection psum is DMA'd straight to DRAM; the fp8 chunks'
        # 2^13 scale is divided out on the host.
        bf = j == 0
        def f():
            ps = pp.tile([128, CH], F32, tag="aux", bufs=1, name="pjo")
            if bf:
                for pr in range(2):
                    for sl in range(2):
                        nc.tensor.matmul(
                            ps, lhsT=wo16[:, pr, sl, 128 * n:128 * (n + 1)],
                            rhs=attnT16[:, pr, sl, :],
                            start=(pr == 0 and sl == 0),
                            stop=(pr == 1 and sl == 1))
            else:
                cs = slice(j * CH, (j + 1) * CH)
                for pr in range(2):
                    nc.tensor.matmul(
                        ps, lhsT=wo8[:, pr, :, 128 * n:128 * (n + 1)],
                        rhs=attnT8[:, pr, :, cs],
                        start=(pr == 0), stop=(pr == 1), perf_mode=DR)
            osb = opool.tile([128, CH], F32, tag="osb", name="osb")
            if bf:
                nc.vector.tensor_copy(osb, ps)
            else:
                nc.vector.tensor_scalar(out=osb, in0=ps, scalar1=OCP,
                                        scalar2=None, op0=mybir.AluOpType.mult)
            nc.sync.dma_start(
                out=outT[128 * n:128 * (n + 1), j * CH:(j + 1) * CH],
                in_=osb)
        return f

    # ---- main schedule ----
    def q8dst_fn(q8t):
        return lambda hg, sl, cs: q8t[:, hg, sl, cs]

    def q8dst_fn2(q8t):
        return lambda hg, sl, cs: q8t[:, hg, sl, :]

    def kdst_fn(hg, sl, cs):
        return kT8[hg][:, sl, cs]

    q8_c0 = qpool.tile([128, 2, 2, CH], F8, tag="q8", name="q8_0")
    for hg in range(2):
        for sl in range(2):
            fillers.append((f"k8c0_{hg}{sl}", kproj8_c0(hg, sl)))
            fillers.append((f"q8c0_{hg}{sl}",
                            proj8(wq8, q8dst_fn(q8_c0), 0, hg, sl,
                                  cs=slice(256, CH))))
        if hg == 0:
            fillers.append(("q16_0", proj16q(0, wq16, q16)))
            fillers.append(("k16_0", proj16q(0, wk16, kT16)))
    fillers.append(("v16_0", vproj16(0)))
    fillers.append(("v16_1", vproj16(1)))
    fillers.append(("v8p_0_2", vproj8(0, 2)))
    fillers.append(("v8p_0_3", vproj8(0, 3)))
    fillers.append(("q16_1", proj16q(1, wq16, q16)))
    fillers.append(("k16_1", proj16q(1, wk16, kT16)))
    fillers.append(("q16_2", proj16q(2, wq16, q16)))
    fillers.append(("k16_2", proj16q(2, wk16, kT16)))
    fillers.append(("q16_3", proj16q(3, wq16, q16)))
    fillers.append(("k16_3", proj16q(3, wk16, kT16)))

    prev_j = None     # chunk awaiting its O-projection
    pend_tail = [None]  # deferred PV+norm of the previous head
    q8_next = None
    q8_cur = q8_c0
    for j in range(NCH):
        if j + 1 < NCH:
            q8_next = qpool.tile([128, 2, 2, CH], F8, tag="q8",
                                 name=f"q8_{j + 1}")
            ptag = "sco" if j <= 1 else "aux"
            pitems = []
            for hg in range(2):
                for sl in range(2):
                    pitems.append((f"q8p_{j+1}_{hg}{sl}",
                                   proj8(wq8, q8dst_fn2(q8_next), j + 1,
                                         hg, sl, tag=ptag)))
                    pitems.append((f"k8p_{j+1}_{hg}{sl}",
                                   proj8(wk8, kdst_fn, j + 1, hg, sl,
                                         tag=ptag)))
            for sb in range(4):
                pitems.append((f"v8p_{j+1}_{sb}", vproj8(j + 1, sb,
                                                         tag=ptag)))
            if j == 0:
                fillers.extend(pitems)
            else:
                for it in reversed(pitems):
                    fillers.appendleft(it)
        attn16 = apool.tile([128, W, HPC, DK], BF16, tag="attn16",
                            name=f"attn16_{j}")
        if prev_j is not None:
            for n in range(8):
                bg.append((f"o_{prev_j}_{n}", oproj_fn(prev_j, n)))
        if j >= 1:
            for hg in range(2):
                for sl in range(2):
                    need(f"q8p_{j}_{hg}{sl}")
                    need(f"k8p_{j}_{hg}{sl}")
        emit_chunk(j, q8ch=q8_cur)
        prev_j = j
        q8_cur = q8_next
        # ensure all stragglers (e.g. v-projections) are in before next chunk
        if j + 1 < NCH:
            need(f"v8p_{j+1}_3")

    if pend_tail[0] is not None:
        pend_tail[0]()
        pend_tail[0] = None
    while fillers or bg:
        pop_filler()

    def oproj2_tail(n2):
        # last-chunk O-projection, 2 d-blocks per sc-tagged psum tile;
        # output in bf16 (halves the tail DMA; <0.2% quantization)
        j = prev_j
        ps = pp.tile([128, 2, CH], F32, tag="sc", name="pot")
        cs = slice(j * CH, (j + 1) * CH)
        for t in range(2):
            n = 2 * n2 + t
            for pr in range(2):
                nc.tensor.matmul(
                    ps[:, t, :], lhsT=wo8[:, pr, :, 128 * n:128 * (n + 1)],
                    rhs=attnT8[:, pr, :, cs],
                    start=(pr == 0), stop=(pr == 1), perf_mode=DR)
        osb = opool.tile([128, 2, CH], BF16, tag="osb2", name="osb2")
        if n2 % 2:
            nc.vector.tensor_scalar(out=osb, in0=ps, scalar1=OCP,
                                    scalar2=None, op0=mybir.AluOpType.mult)
        else:
            nc.scalar.activation(out=osb, in_=ps,
                                 func=mybir.ActivationFunctionType.Copy,
                                 scale=OCP)
        eng = nc.sync if n2 % 2 else nc.gpsimd
        dst = outT16[256 * n2:256 * (n2 + 1), :].rearrange(
            "(t p) c -> p t c", t=2)
        eng.dma_start(out=dst, in_=osb)

    for n2 in range(4):
        oproj2_tail(n2)


_CACHE = {}


def _get_nc():
    if "nc" in _CACHE:
        return _CACHE["nc"]
    tile.TileContext._drain_and_barrier = _drain_and_barrier_split
    nc = bass.Bass("TRN2", target_bir_lowering=False, debug=False)
    din = {
        "x8": nc.dram_tensor("x8", [128, 4, 2, S], F8,
                             kind="ExternalInput").ap(),
        "x16": nc.dram_tensor("x16", [128, 4, 2, 256], BF16,
                              kind="ExternalInput").ap(),
        "wq8": nc.dram_tensor("wq8", [128, 4, 2, 2, 2, 128], F8,
                              kind="ExternalInput").ap(),
        "wk8": nc.dram_tensor("wk8", [128, 4, 2, 2, 2, 128], F8,
                              kind="ExternalInput").ap(),
        "wv8": nc.dram_tensor("wv8", [128, 4, 2, 512], F8,
                              kind="ExternalInput").ap(),
        "wo8": nc.dram_tensor("wo8", [128, 2, 2, D], F8,
                              kind="ExternalInput").ap(),
        "wq16": nc.dram_tensor("wq16", [128, 4, 4, 2, 128], BF16,
                               kind="ExternalInput").ap(),
        "wk16": nc.dram_tensor("wk16", [128, 4, 4, 2, 128], BF16,
                               kind="ExternalInput").ap(),
        "wv16": nc.dram_tensor("wv16", [128, 4, 2, 512], BF16,
                               kind="ExternalInput").ap(),
        "wo16": nc.dram_tensor("wo16", [128, 2, 2, D], BF16,
                               kind="ExternalInput").ap(),
    }
    outT = nc.dram_tensor("outT", [D, S], F32, kind="ExternalOutput").ap()
    outT16 = nc.dram_tensor("outT16", [D, CH], BF16,
                            kind="ExternalOutput").ap()
    from contextlib import ExitStack
    with tile.TileContext(nc) as tc, ExitStack() as ctx:
        _build_kernel(ctx, tc, din, outT, outT16)
    _split_excess_waits(nc)
    _CACHE["nc"] = nc
    return nc


def make_in_maps(x, Wq, Wk, Wv, Wo):
    x = np.asarray(x, np.float32)
    Wq, Wk, Wv, Wo = (np.asarray(w, np.float32) for w in (Wq, Wk, Wv, Wo))
    xb8, xb16 = [], []
    for b in range(B):
        xT = np.ascontiguousarray(x[b].T)                      # [D, S]
        x4 = xT.reshape(4, 2, 128, S).transpose(2, 0, 1, 3)    # [128,4,2,S]
        xb8.append(np.ascontiguousarray((x4 * XS)).astype(NPF8))
        xb16.append(np.ascontiguousarray(x4[:, :, :, :256]).astype(NPBF))

    def pack_dr_qk(Wm, hh):
        # [128p, 4pr, 2csl, 2hg, 2qsl, 128(h*32+dk)]
        Wc = Wm[512 * hh:512 * hh + 512, :]
        a = Wc.reshape(2, 4, 2, 32, 4, 2, 128)   # hg,h,qsl,dk,pr,csl,p
        a = a.transpose(6, 4, 5, 0, 2, 1, 3)     # p,pr,csl,hg,qsl,h,dk
        return np.ascontiguousarray(a.reshape(128, 4, 2, 2, 2, 128))

    def pack_16_qk(Wm, hh):
        # [128p, 4mb, 4pr, 2csl, 128(a*64+dk)]
        Wc = Wm[512 * hh:512 * hh + 512, :]
        a = Wc.reshape(4, 2, 64, 4, 2, 128)      # mb,a,dk,pr,csl,p
        a = a.transpose(5, 0, 3, 4, 1, 2)        # p,mb,pr,csl,a,dk
        return np.ascontiguousarray(a.reshape(128, 4, 4, 2, 128))

    def pack_v(Wm, hh):
        # [128p, 4pr, 2csl, 512(h*64+dv)]
        Wc = Wm[512 * hh:512 * hh + 512, :]
        a = Wc.reshape(8, 64, 4, 2, 128)         # h,dv,pr,csl,p
        a = a.transpose(4, 2, 3, 0, 1)           # p,pr,csl,h,dv
        return np.ascontiguousarray(a.reshape(128, 4, 2, 512))

    def pack_o(Wm, hh):
        # [128p, 2pr, 2sl, 1024n]
        Wc = Wm[:, 512 * hh:512 * hh + 512].T    # [512 hd, 1024 n]
        a = Wc.reshape(2, 2, 128, D)             # pr,sl,p,n
        return np.ascontiguousarray(a.transpose(2, 0, 1, 3))

    packs = []
    for hh in range(2):
        packs.append({
            "wq8": (pack_dr_qk(Wq, hh) * WS).astype(NPF8),
            "wk8": (pack_dr_qk(Wk, hh) * WS).astype(NPF8),
            "wv8": (pack_v(Wv, hh) * WS).astype(NPF8),
            "wo8": (pack_o(Wo, hh) * WS).astype(NPF8),
            "wq16": pack_16_qk(Wq, hh).astype(NPBF),
            "wk16": pack_16_qk(Wk, hh).astype(NPBF),
            "wv16": pack_v(Wv, hh).astype(NPBF),
            "wo16": pack_o(Wo, hh).astype(NPBF),
        })

    in_maps = []
    for c in range(8):
        b, hh = c // 2, c % 2
        m = {"x8": xb8[b], "x16": xb16[b]}
        m.update(packs[hh])
        in_maps.append(m)
    return in_maps


def kernel(x, Wq, Wk, Wv, Wo, _trace=False, _trace_kwargs=None):
    nc = _get_nc()
    in_maps = make_in_maps(x, Wq, Wk, Wv, Wo)
    res = run_bass_kernel_spmd(
        nc, in_maps, core_ids=list(range(8)), trace=_trace,
        **(_trace_kwargs or {}))
    outs = []
    for c in range(8):
        o = res.results[c]["outT"].copy()
        o[:, 3 * CH:] = res.results[c]["outT16"].astype(np.float32)
        outs.append(o)
    full = np.stack([(outs[2 * b] + outs[2 * b + 1]).T for b in range(B)])
    if _trace:
        _CACHE["last_results"] = res
    return full.astype(np.float32)


# revision 47
# speedup vs baseline: 1.5714x; 1.0121x over previous
"""Causal multi-head self-attention on 8 Trainium2 NeuronCores.

Problem: x[4,2048,1024], Wq/Wk/Wv/Wo[1024,1024], H=16 heads, dk=64.
  q = x@Wq.T, k = x@Wk.T, v = x@Wv.T  (per-head causal softmax(q k^T/8) v) @ Wo.T

Sharding: core c handles batch b=c//2 and head-half hh=c%2 (8 heads).
Each core returns a partial transposed output outT[D,S] (its 512 attn
columns through the matching 512 rows of Wo.T); the host sums core
pairs and transposes.

Precision plan (rel-err budget 2e-2; lands ~5e-3):
  chunk 0 (q rows 0..511)   : bf16 everywhere (early rows have small
                              softmax support -> errors don't average).
  chunks 1-3 (rows 512..2047): fp8e4m3 with MatmulPerfMode.DoubleRow
                              (0.5 cycles/output column, 2x PE rate).
Host pre-quantizes and pre-packs x and all weights into the exact SBUF
tile layouts (including DoubleRow pair/slot packing and head
permutations), so the kernel DMAs everything linearly.

Kernel structure per chunk:
  scores^T [k,q] via DR matmuls (q/k stored [32-part, 2 dk-slot, s],
  4 heads per tile on PE quadrants 0/32/64/96 via explicit
  tile_position); exp on ACT (scale folds the 16*16 fp8 scaling,
  bias ln4 rescales e into fp8 range) writing fp8 e-tiles directly;
  causal masking = column trim + gpsimd affine_select zero-fill;
  PV reoriented as e.T@v -> at[q, 65] (65-column outputs; ones column
  of v gives the denominator; DR pairs 2 k-blocks per matmul);
  normalization = DVE reciprocal + per-partition tensor_scalar (the
  [q,dv] orientation makes the denominator a per-partition scalar);
  attn transposed back to [hd,q] with bf16 PE-transposes (2 heads per
  128x128 transpose) for the DoubleRow O-projection, which emits
  outT[d,q] tiles DMA'd to a transposed DRAM output.

exp on ACT is the bottleneck (~16.8M causal score elements per core at
1 elem/cycle/partition); PE work is interleaved into the exp shadow
via a filler deque (next chunk's projections, previous chunk's
transposes + O-projection).
"""

import numpy as np
import ml_dtypes

import concourse.bass as bass
import concourse.mybir as mybir
import concourse.tile as tile
from concourse.bass_utils import run_bass_kernel_spmd
from concourse.vector_clock import ScopedClock, VectorClock
from collections import deque

B, S, D, H, DK = 4, 2048, 1024, 16, 64
HPC = H // 2          # heads per core
CH = 512              # q-chunk width
NCH = S // CH         # 4
W = CH // 128         # q-windows per chunk (4)
F32 = mybir.dt.float32
I32 = mybir.dt.int32
F32R = mybir.dt.float32r
BF16 = mybir.dt.bfloat16
F8 = mybir.dt.float8e4
DR = mybir.MatmulPerfMode.DoubleRow
EXP = mybir.ActivationFunctionType.Exp
LN4 = float(np.log(4.0))
NPF8 = ml_dtypes.float8_e4m3
NPBF = ml_dtypes.bfloat16

XS, WS = 8.0, 256.0   # host scales: x8 = 8x, w8 = 256W
# Schraudolph exp2 bit-trick constants for offloaded exps (DVE+Pool):
# i32 = round(sc * 2^-11 * log2e * 2^23 + (129 - cadj) * 2^23); bitcast f32
# gives ~4*exp(sc/2048) with ~3% piecewise-linear error.
EXA = float(np.float32(1.4426950408889634 * (1 << 23) * 2.0 ** -11))
EXB = float(np.float32((129.0 - 0.044) * (1 << 23)))
# fp8-projection psum = 2048*val -> q8/k8 stored 16*val, v8 stored 32*val
QCP = 2.0 ** -7
VCP = 2.0 ** -6
OCP = 2.0 ** -13      # O-proj psum (32*256=8192) -> out


def _drain_and_barrier_split(self, tick_clock, wait_clock):
    # The stock Tile tail drain attaches every outstanding sem wait to one
    # Drain instruction; this walrus build caps sync waits per instruction
    # and rejects it.  Put each wait on its own SP nop first, then drain
    # with no waits (SP has observed everything by then).
    gc = tick_clock.global_clock
    n = len(gc)
    for proc in range(n):
        t = gc[proc]
        if t == 0:
            continue
        vc = VectorClock([0] * n)
        vc.require_at_least(proc, t)
        nop = self.nc.sync.nop(nofuse=True)
        wait_clock.add_sem_waits(nop.ins, ScopedClock({None: vc}))
    self.nc.sync.drain()
    self.nc.all_engine_barrier()
    assert self.sems is not None
    popped = self.nc._tile_sem_poison_stack.pop()
    assert popped is self._sem_poison
    self.nc.clear_and_free_semaphores(list(self.sems.allocated().values()))
    self.nc.all_engine_barrier()


def _split_excess_waits(nc, max_waits=1):
    # This walrus build rejects instructions carrying more than a couple of
    # sem waits ("Too many sync wait commands").  Engines execute their
    # stream in order, so excess waits can be moved onto nofuse nops placed
    # immediately before the instruction on the same engine.
    ctr = 0
    for blk in nc.m.functions[0].blocks:
        insts = blk.instructions
        out = []
        changed = False
        for inst in insts:
            si = inst.sync_info
            if si is not None and si.on_wait and len(si.on_wait) > max_waits:
                waits = list(si.on_wait)
                extra, keep = waits[:-max_waits], waits[-max_waits:]
                for gi in range(0, len(extra), max_waits):
                    ctr += 1
                    out.append(mybir.InstNoOp(
                        name=f"wsplit_{ctr}",
                        engine=inst.engine,
                        bass_nofuse=True,
                        sync_info=mybir.SyncInfo(
                            on_wait=extra[gi:gi + max_waits], on_update=[]),
                    ))
                inst.sync_info = mybir.SyncInfo(
                    on_wait=keep, on_update=si.on_update)
                changed = True
            out.append(inst)
        if changed:
            insts[:] = out


def _build_kernel(ctx, tc, din, outT, outT16):
    nc = tc.nc

    wpool = ctx.enter_context(tc.tile_pool(name="weights", bufs=1))
    kvpool = ctx.enter_context(tc.tile_pool(name="kv", bufs=1))
    qpool = ctx.enter_context(tc.tile_pool(name="q", bufs=2))
    epool = ctx.enter_context(tc.tile_pool(name="exp", bufs=16))
    apool = ctx.enter_context(tc.tile_pool(name="attn", bufs=2))
    opool = ctx.enter_context(tc.tile_pool(name="osb", bufs=4))
    rpool = ctx.enter_context(tc.tile_pool(name="recip", bufs=2))
    # PSUM, 8 banks: sc 2x[128,2,512]f32 (4: ACT-consumed score ring)
    #   + sco 2x[128,512]f32 (2: DVE-consumed offloaded-score ring)
    #   + at 1x[128,4,128]f32 (1: PV accumulator; single-buffered thanks to
    #     the deferred head-tail) + aux 1x (1: fillers/transposes/O-proj)
    pp = ctx.enter_context(tc.tile_pool(name="pp", bufs=2, space="PSUM"))

    # ---- persistent tiles ----
    x8 = wpool.tile([128, 4, 2, S], F8, tag="x8")
    x16 = wpool.tile([128, 4, 2, 256], BF16, tag="x16")
    wq8 = wpool.tile([128, 4, 2, 2, 2, 128], F8, tag="wq8")
    wk8 = wpool.tile([128, 4, 2, 2, 2, 128], F8, tag="wk8")
    wv8 = wpool.tile([128, 4, 2, 512], F8, tag="wv8")
    wo8 = wpool.tile([128, 2, 2, D], F8, tag="wo8")
    wq16 = [wpool.tile([128, 4, 2, 128], BF16, tag=f"wq16_{mb}",
                       name=f"wq16_{mb}") for mb in range(4)]
    wk16 = [wpool.tile([128, 4, 2, 128], BF16, tag=f"wk16_{mb}",
                       name=f"wk16_{mb}") for mb in range(4)]
    wv16 = wpool.tile([128, 4, 2, 512], BF16, tag="wv16")
    wo16 = wpool.tile([128, 2, 2, D], BF16, tag="wo16")

    kT8 = [kvpool.tile([128, 2, S], F8, tag=f"kT8_{hg}", name=f"kT8_{hg}")
           for hg in range(2)]
    kT16 = kvpool.tile([128, 4, 256], BF16, tag="kT16")
    q16 = kvpool.tile([128, 4, 256], BF16, tag="q16")
    v8 = kvpool.tile([128, 8, 2, HPC, DK + 1], F8, tag="v8")
    v16 = kvpool.tile([128, 2, 2, HPC, DK + 1], BF16, tag="v16")
    attnT8 = kvpool.tile([128, 2, 2, S], F8, tag="attnT8")
    attnT16 = kvpool.tile([128, 2, 2, CH], BF16, tag="attnT16")

    ident = wpool.tile([128, 128], BF16, tag="ident")
    ln4ap = wpool.tile([128, 1], F32, tag="ln4")
    warm = wpool.tile([128, 128], F32R, tag="warm")

    # ---- input DMAs (priority order on the sync queue) ----
    nc.sync.dma_start(out=wk8, in_=din["wk8"])
    nc.sync.dma_start(out=wq8, in_=din["wq8"])
    csq = slice(0, CH)
    nc.sync.dma_start(out=x8[:, :, :, csq], in_=din["x8"][:, :, :, csq])
    for pr in range(4):
        nc.sync.dma_start(out=x16[:, pr], in_=din["x16"][:, pr])
    nc.sync.dma_start(out=wq16[0], in_=din["wq16"][:, 0])
    nc.sync.dma_start(out=wk16[0], in_=din["wk16"][:, 0])
    nc.sync.dma_start(out=wv16, in_=din["wv16"])
    nc.sync.dma_start(out=wv8, in_=din["wv8"])
    csb = slice(CH, 2 * CH)
    nc.sync.dma_start(out=x8[:, :, :, csb], in_=din["x8"][:, :, :, csb])
    for mb in range(1, 4):
        nc.sync.dma_start(out=wq16[mb], in_=din["wq16"][:, mb])
        nc.sync.dma_start(out=wk16[mb], in_=din["wk16"][:, mb])
    csr = slice(2 * CH, S)
    nc.sync.dma_start(out=x8[:, :, :, csr], in_=din["x8"][:, :, :, csr])
    nc.sync.dma_start(out=wo8, in_=din["wo8"])
    nc.sync.dma_start(out=wo16, in_=din["wo16"])

    # ---- init + warmup (under input DMAs) ----
    warm_f = wpool.tile([128, 128], F32, tag="warm_f")
    nc.gpsimd.memset(warm_f, 0.0)
    nc.gpsimd.tensor_copy(warm, warm_f)
    tmpf = wpool.tile([128, 128], F32, tag="tmpf")
    nc.vector.memset(tmpf, 1.0)
    nc.gpsimd.affine_select(out=tmpf, in_=tmpf,
                            compare_op=mybir.AluOpType.is_equal, fill=0.0,
                            base=0, channel_multiplier=-1, pattern=[[1, 128]])
    nc.vector.tensor_copy(ident, tmpf)
    nc.vector.memset(ln4ap, LN4)
    nc.vector.memset(v8[:, :, :, :, DK:DK + 1], 1.0)
    nc.vector.memset(v16[:, :, :, :, DK:DK + 1], 1.0)
    # preload the ACT exp table set early
    etw = rpool.tile([128, 16], F32, tag="etw", bufs=1)
    nc.scalar.activation(out=etw, in_=tmpf[:, 0:16], func=EXP, scale=1.0)
    # hold the PE clock-gate open / absorb the cold ramp while DMAs land
    wps = pp.tile([128, 2, CH], F32, tag="sc", name="wps")
    for r in range(26):
        nc.tensor.matmul(wps[:, r % 2, 0:128], lhsT=warm, rhs=warm,
                         start=True, stop=True)

    # ---- chunk-0 bf16 projections ----
    # q16/kT16: baseline row layout (mb-block = 2 heads x 64 dk).
    def proj16q(mb, wtiles, dst, tag="sco"):
        def g():
            ps = pp.tile([128, 256], F32, tag=tag, bufs=None if tag == "sc"
                         else 2, name="pq16")
            for pr in range(4):
                for csl in range(2):
                    nc.tensor.matmul(
                        ps, lhsT=wtiles[mb][:, pr, csl, :],
                        rhs=x16[:, pr, csl, :],
                        start=(pr == 0 and csl == 0),
                        stop=(pr == 3 and csl == 1))
            nc.vector.tensor_copy(dst[:, mb, :], ps)
        return g

    def vproj16(sb):
        # v16 s-block sb (+ fp8 copy for later chunks)
        def f():
            ps = pp.tile([128, CH], F32, tag="sco", bufs=2, name="pv16")
            for pr in range(4):
                for csl in range(2):
                    nc.tensor.matmul(
                        ps, lhsT=x16[:, pr, csl, sb * 128:(sb + 1) * 128],
                        rhs=wv16[:, pr, csl, :],
                        start=(pr == 0 and csl == 0),
                        stop=(pr == 3 and csl == 1))
            pv = ps.rearrange("p (h d) -> p h d", h=HPC)
            nc.vector.tensor_copy(v16[:, sb // 2, sb % 2, :, 0:DK], pv)
            nc.vector.tensor_scalar(
                out=v8[:, sb // 2, sb % 2, :, 0:DK], in0=pv, scalar1=32.0,
                scalar2=None, op0=mybir.AluOpType.mult)
        return f

    def kproj8_c0(hg, sl):
        # fp8 K-projection of chunk-0 columns into kT8 (DR layout)
        def f():
            ps = pp.tile([128, CH], F32, tag="sco", bufs=2, name="pk8c0")
            for pr in range(4):
                nc.tensor.matmul(
                    ps, lhsT=wk8[:, pr, :, hg, sl, :],
                    rhs=x8[:, pr, :, 0:CH],
                    start=(pr == 0), stop=(pr == 3), perf_mode=DR)
            nc.vector.tensor_scalar(
                out=kT8[hg][:, sl, 0:CH], in0=ps, scalar1=QCP,
                scalar2=None, op0=mybir.AluOpType.mult)
        return f

    # ---- fp8 projections for chunk j>=1 ----
    def proj8(wsb, dstf, j, hg, sl, cs=None, tag="sco"):
        # q8/kT8 [4 heads x 32] dk-slot sl over chunk-j columns
        if cs is None:
            cs = slice(j * CH, (j + 1) * CH)
        def f():
            ps = pp.tile([128, cs.stop - cs.start], F32, tag=tag,
                         bufs=2 if tag == "sco" else 1, name="p8")
            for pr in range(4):
                nc.tensor.matmul(
                    ps, lhsT=wsb[:, pr, :, hg, sl, :],
                    rhs=x8[:, pr, :, cs],
                    start=(pr == 0), stop=(pr == 3), perf_mode=DR)
            nc.vector.tensor_scalar(
                out=dstf(hg, sl, cs), in0=ps, scalar1=QCP, scalar2=None,
                op0=mybir.AluOpType.mult)
        return f

    def vproj8(j, sb, tag="sco"):
        # v8 s-block 4j+sb
        def f():
            ps = pp.tile([128, CH], F32, tag=tag,
                         bufs=2 if tag == "sco" else 1, name="pv8")
            blk = 4 * j + sb
            for pr in range(4):
                nc.tensor.matmul(
                    ps, lhsT=x8[:, pr, :, blk * 128:(blk + 1) * 128],
                    rhs=wv8[:, pr, :, :],
                    start=(pr == 0), stop=(pr == 3), perf_mode=DR)
            nc.vector.tensor_scalar(
                out=v8[:, blk // 2, blk % 2, :, 0:DK],
                in0=ps.rearrange("p (h d) -> p h d", h=HPC),
                scalar1=VCP, scalar2=None, op0=mybir.AluOpType.mult)
        return f

    # ---- filler machinery ----
    fillers = deque()    # (label, fn): deadline-ordered work
    bg = deque()         # no-deadline work (O-projections)
    done = set()

    def pop_filler(n=1):
        for _ in range(n):
            if fillers:
                label, f = fillers.popleft()
                f()
                done.add(label)
            elif bg:
                label, f = bg.popleft()
                f()
                done.add(label)
            else:
                return

    def need(label):
        while label not in done and fillers:
            lb, f = fillers.popleft()
            f()
            done.add(lb)

    # ---- per-chunk attention ----
    def emit_chunk(j, q8ch):
        """Attention for chunk j.  q8ch: [128, 2hg, 2sl, CH] fp8 q tile
        (chunk 0 only holds columns 256:512; its windows 0-1 run on the
        bf16 q16/kT16/v16 path)."""
        bf = j == 0
        for h in range(HPC):
            if bf and h >= 2:
                need(f"q16_{h // 2}")
                need(f"k16_{h // 2}")
            at = pp.tile([128, W, 128], F32, tag="at", bufs=1,
                         name=f"at{j}_{h}")
            npairs = 2 * (j + 1)
            es = [None] * npairs

            def qlo(i):
                return max(0, 128 * i - CH * j)

            def emit_pv_all(h=h, at=at, es=es):
                # One window at a time: a start=True marks the whole PSUM
                # bank pending-zero, so windows must be accumulated fully
                # before the next window's group begins.
                for w in range(W):
                    gmax = (4 * j + w) // 2
                    if bf and w < 2:
                        e16 = es[0][0]
                        for t in range(w + 1):
                            nc.tensor.matmul(
                                at[:, w, 0:DK + 1],
                                lhsT=e16[:, t, 128 * w:128 * (w + 1)],
                                rhs=v16[:, 0, t, h, :],
                                start=(t == 0), stop=(t == w),
                                skip_group_check=True)
                        continue
                    for g in range(gmax + 1):
                        e8 = es[g][1] if bf else es[g]
                        nc.tensor.matmul(
                            at[:, w, 0:DK + 1],
                            lhsT=e8[:, :, 128 * w:128 * (w + 1)],
                            rhs=v8[:, g, :, h, :],
                            start=(g == 0), stop=(g == gmax),
                            perf_mode=DR, skip_group_check=True)

            gorder = list(range(npairs))
            if j == NCH - 1 and h == HPC - 1:
                # last head: diagonal (masked) pairs first so the final
                # PV burst doesn't wait on late pool-side masks
                gorder = gorder[-2:] + gorder[:-2]
            for g in gorder:
                qlp = qlo(2 * g)
                offl = ((j == 2 and g == 1)
                        or (j == 3 and g in ((3, 5) if h == 0 else (1, 3, 5))
                            and not (h == HPC - 1 and g == 5)))
                if offl:
                    sco = [pp.tile([128, CH], F32, tag="sco", bufs=2,
                                   name=f"sco{t}") for t in range(2)]
                    sc = None
                else:
                    sc = pp.tile([128, 2, CH], F32, tag="sc", name="sc")
                # fp8 scores for columns [max(ql,256*bf):512)
                hg, hq = h // 4, h % 4
                if bf and g == 0:
                    for sl in range(2):
                        need(f"k8c0_{hg}{sl}")
                        need(f"q8c0_{hg}{sl}")
                rq = slice(32 * hq, 32 * hq + 32)
                for t in range(2):
                    i = 2 * g + t
                    ql = max(qlo(i), 256) if bf else qlo(i)
                    nc.tensor.matmul(
                        sco[t][:, ql:] if offl else sc[:, t, ql:],
                        lhsT=kT8[hg][rq, :, 128 * i:128 * (i + 1)],
                        rhs=q8ch[rq, hg, :, ql:],
                        start=True, stop=True, perf_mode=DR,
                        tile_position=(32 * hq, 0))
                if bf:
                    e8 = epool.tile([128, 2, CH], F8, tag="e8",
                                    name=f"e8_{j}_{h}_{g}")
                    q8l = max(qlp, 256)
                    nc.scalar.activation(
                        out=e8[:, :, q8l:], in_=sc[:, :, q8l:], func=EXP,
                        scale=2.0 ** -11, bias=ln4ap[:, 0:1])
                    for t in range(2):
                        i = 2 * g + t
                        hi8 = min(128 * i + 128, CH)
                        if hi8 > q8l:
                            nc.gpsimd.affine_select(
                                out=e8[:, t, q8l:hi8],
                                in_=e8[:, t, q8l:hi8],
                                compare_op=mybir.AluOpType.is_ge,
                                fill=0.0, base=q8l - i * 128,
                                channel_multiplier=-1,
                                pattern=[[1, hi8 - q8l]])
                    es[g] = (None, e8)
                else:
                    e = epool.tile([128, 2, CH], F8, tag="e8",
                                   name=f"e{j}_{h}_{g}")
                    if offl:
                        # offload to DVE (exp2 bit-trick) + Pool (fp8 cast)
                        i32 = epool.tile([128, 2, CH], I32, tag="i32",
                                         bufs=4, name=f"i{j}_{h}_{g}")
                        for t in range(2):
                            nc.vector.tensor_scalar(
                                out=i32[:, t, :], in0=sco[t], scalar1=EXA,
                                scalar2=EXB, op0=mybir.AluOpType.mult,
                                op1=mybir.AluOpType.add)
                        nc.gpsimd.tensor_copy(e, i32.bitcast(F32))
                    else:
                        nc.scalar.activation(
                            out=e[:, :, qlp:], in_=sc[:, :, qlp:], func=EXP,
                            scale=2.0 ** -11, bias=ln4ap[:, 0:1])
                    for t in range(2):
                        i = 2 * g + t
                        if i >= 4 * j:
                            hi = min(128 * (i - 4 * j) + 128, CH)
                            if hi > qlp:
                                nc.gpsimd.affine_select(
                                    out=e[:, t, qlp:hi], in_=e[:, t, qlp:hi],
                                    compare_op=mybir.AluOpType.is_ge,
                                    fill=0.0, base=j * CH + qlp - i * 128,
                                    channel_multiplier=-1,
                                    pattern=[[1, hi - qlp]])
                    es[g] = e
                if g == gorder[1] and pend_tail[0] is not None:
                    pend_tail[0]()
                    pend_tail[0] = None
                if j < 2:
                    pop_filler(2 if len(fillers) > 10 else 1)
                elif g not in gorder[:2]:
                    pop_filler(1)

            if bf:
                # bf16 scores + exp for columns [0:256) (windows 0-1),
                # emitted after the head's fp8 stream so the late-arriving
                # x16/wq16 DMAs never stall the fp8 pipeline
                need("q16_0")
                need("k16_0")
                mb, a = h // 2, h % 2
                row = slice(a * DK, (a + 1) * DK)
                sc16 = pp.tile([128, 2, 256], F32, tag="sco", bufs=2,
                               name="sc16")
                for t in range(2):
                    ql = qlo(t)
                    nc.tensor.matmul(
                        sc16[:, t, ql:],
                        lhsT=kT16[row, mb, 128 * t:128 * (t + 1)],
                        rhs=q16[row, mb, ql:],
                        start=True, stop=True)
                e16 = epool.tile([128, 2, 256], BF16, tag="e16",
                                 name=f"e16_{h}")
                nc.scalar.activation(
                    out=e16, in_=sc16, func=EXP,
                    scale=0.125, bias=ln4ap[:, 0:1])
                for t in range(2):
                    hi16 = min(128 * t + 128, 256)
                    nc.gpsimd.affine_select(
                        out=e16[:, t, 0:hi16], in_=e16[:, t, 0:hi16],
                        compare_op=mybir.AluOpType.is_ge,
                        fill=0.0, base=-t * 128,
                        channel_multiplier=-1, pattern=[[1, hi16]])
                es[0] = (e16, es[0][1])

            def head_tail(h=h, at=at, es=es, attn16_t=attn16,
                          emit_pv_all=emit_pv_all):
                if bf:
                    need("v16_0")
                    need("v16_1")
                    need("v8p_0_2")
                    need("v8p_0_3")
                emit_pv_all()
                # normalization: recip of the ones-column, then one
                # broadcast multiply over all four windows
                rc = rpool.tile([128, W, 1], F32, tag="rc",
                                name=f"rc{j}_{h}")
                with nc.allow_low_precision(
                        reason="softmax denominator recip"):
                    nc.vector.reciprocal(out=rc[:, :, 0], in_=at[:, :, DK])
                if bf:
                    # windows 2-3 carry the fp8 32x scale; fold 1/32 into rc
                    nc.vector.tensor_scalar(
                        out=rc[:, 2:4, 0], in0=rc[:, 2:4, 0],
                        scalar1=2.0 ** -5, scalar2=None,
                        op0=mybir.AluOpType.mult)
                nc.vector.tensor_tensor(
                    out=attn16_t[:, :, h, :], in0=at[:, :, 0:DK],
                    in1=rc.to_broadcast([128, W, DK]),
                    op=mybir.AluOpType.mult)
                if h % 2 == 1:
                    # transpose this head pair once both are normalized
                    fillers.append((f"tr_{j}_{h // 2}",
                                    transpose_fn(j, h // 2, attn16_t)))
            if pend_tail[0] is not None:
                pend_tail[0]()
            pend_tail[0] = head_tail

    def transpose_fn(j, hp, attn16_t):
        bf = j == 0
        def f():
            tr = pp.tile([128, W, 256], BF16, tag="aux", bufs=1, name="tr")
            for w in range(W):
                nc.tensor.matmul(
                    tr[:, w, 0:128],
                    lhsT=attn16_t[:, w, 2 * hp:2 * hp + 2, :],
                    rhs=ident, is_transpose=True, start=True, stop=True)
            dst = attnT16 if bf else attnT8
            cs = slice(0, CH) if bf else slice(j * CH, (j + 1) * CH)
            dview = dst[:, hp // 2, hp % 2, cs].rearrange(
                "p (a b) -> p a b", a=W)
            nc.vector.tensor_copy(dview, tr[:, :, 0:128])
        return f

    def oproj_fn(j, n):
        # O-projection psum is DMA'd straight to DRAM; the fp8 chunks'
        # 2^13 scale is divided out on the host.
        bf = j == 0
        def f():
            ps = pp.tile([128, CH], F32, tag="aux", bufs=1, name="pjo")
            if bf:
                for pr in range(2):
                    for sl in range(2):
                        nc.tensor.matmul(
                            ps, lhsT=wo16[:, pr, sl, 128 * n:128 * (n + 1)],
                            rhs=attnT16[:, pr, sl, :],
                            start=(pr == 0 and sl == 0),
                            stop=(pr == 1 and sl == 1))
            else:
                cs = slice(j * CH, (j + 1) * CH)
                for pr in range(2):
                    nc.tensor.matmul(
                        ps, lhsT=wo8[:, pr, :, 128 * n:128 * (n + 1)],
                        rhs=attnT8[:, pr, :, cs],
                        start=(pr == 0), stop=(pr == 1), perf_mode=DR)
            osb = opool.tile([128, CH], F32, tag="osb", name="osb")
            if bf:
                nc.vector.tensor_copy(osb, ps)
            else:
                nc.vector.tensor_scalar(out=osb, in0=ps, scalar1=OCP,
                                        scalar2=None, op0=mybir.AluOpType.mult)
            nc.sync.dma_start(
                out=outT[128 * n:128 * (n + 1), j * CH:(j + 1) * CH],
                in_=osb)
        return f

    # ---- main schedule ----
    def q8dst_fn(q8t):
        return lambda hg, sl, cs: q8t[:, hg, sl, cs]

    def q8dst_fn2(q8t):
        return lambda hg, sl, cs: q8t[:, hg, sl, :]

    def kdst_fn(hg, sl, cs):
        return kT8[hg][:, sl, cs]

    q8_c0 = qpool.tile([128, 2, 2, CH], F8, tag="q8", name="q8_0")
    for hg in range(2):
        for sl in range(2):
            fillers.append((f"k8c0_{hg}{sl}", kproj8_c0(hg, sl)))
            fillers.append((f"q8c0_{hg}{sl}",
                            proj8(wq8, q8dst_fn(q8_c0), 0, hg, sl,
                                  cs=slice(256, CH))))
        if hg == 0:
            fillers.append(("q16_0", proj16q(0, wq16, q16)))
            fillers.append(("k16_0", proj16q(0, wk16, kT16)))
    fillers.append(("v16_0", vproj16(0)))
    fillers.append(("v16_1", vproj16(1)))
    fillers.append(("v8p_0_2", vproj8(0, 2)))
    fillers.append(("v8p_0_3", vproj8(0, 3)))
    fillers.append(("q16_1", proj16q(1, wq16, q16)))
    fillers.append(("k16_1", proj16q(1, wk16, kT16)))
    fillers.append(("q16_2", proj16q(2, wq16, q16)))
    fillers.append(("k16_2", proj16q(2, wk16, kT16)))
    fillers.append(("q16_3", proj16q(3, wq16, q16)))
    fillers.append(("k16_3", proj16q(3, wk16, kT16)))

    prev_j = None     # chunk awaiting its O-projection
    pend_tail = [None]  # deferred PV+norm of the previous head
    q8_next = None
    q8_cur = q8_c0
    for j in range(NCH):
        if j + 1 < NCH:
            q8_next = qpool.tile([128, 2, 2, CH], F8, tag="q8",
                                 name=f"q8_{j + 1}")
            ptag = "sco" if j <= 1 else "aux"
            pitems = []
            for hg in range(2):
                for sl in range(2):
                    pitems.append((f"q8p_{j+1}_{hg}{sl}",
                                   proj8(wq8, q8dst_fn2(q8_next), j + 1,
                                         hg, sl, tag=ptag)))
                    pitems.append((f"k8p_{j+1}_{hg}{sl}",
                                   proj8(wk8, kdst_fn, j + 1, hg, sl,
                                         tag=ptag)))
            for sb in range(4):
                pitems.append((f"v8p_{j+1}_{sb}", vproj8(j + 1, sb,
                                                         tag=ptag)))
            if j == 0:
                fillers.extend(pitems)
            else:
                for it in reversed(pitems):
                    fillers.appendleft(it)
        attn16 = apool.tile([128, W, HPC, DK], BF16, tag="attn16",
                            name=f"attn16_{j}")
        if prev_j is not None:
            for n in range(8):
                bg.append((f"o_{prev_j}_{n}", oproj_fn(prev_j, n)))
        if j >= 1:
            for hg in range(2):
                for sl in range(2):
                    need(f"q8p_{j}_{hg}{sl}")
                    need(f"k8p_{j}_{hg}{sl}")
        emit_chunk(j, q8ch=q8_cur)
        prev_j = j
        q8_cur = q8_next
        # ensure all stragglers (e.g. v-projections) are in before next chunk
        if j + 1 < NCH:
            need(f"v8p_{j+1}_3")

    if pend_tail[0] is not None:
        pend_tail[0]()
        pend_tail[0] = None
    while fillers or bg:
        pop_filler()

    def oproj2_tail(n2):
        # last-chunk O-projection, 2 d-blocks per sc-tagged psum tile;
        # output in bf16 (halves the tail DMA; <0.2% quantization)
        j = prev_j
        ps = pp.tile([128, 2, CH], F32, tag="sc", name="pot")
        cs = slice(j * CH, (j + 1) * CH)
        for t in range(2):
            n = 2 * n2 + t
            for pr in range(2):
                nc.tensor.matmul(
                    ps[:, t, :], lhsT=wo8[:, pr, :, 128 * n:128 * (n + 1)],
                    rhs=attnT8[:, pr, :, cs],
                    start=(pr == 0), stop=(pr == 1), perf_mode=DR)
        osb = opool.tile([128, 2, CH], BF16, tag="osb2", name="osb2")
        if n2 % 2:
            nc.vector.tensor_scalar(out=osb, in0=ps, scalar1=OCP,
                                    scalar2=None, op0=mybir.AluOpType.mult)
        else:
            nc.scalar.activation(out=osb, in_=ps,
                                 func=mybir.ActivationFunctionType.Copy,
                                 scale=OCP)
        eng = nc.sync if n2 % 2 else nc.gpsimd
        dst = outT16[256 * n2:256 * (n2 + 1), :].rearrange(
            "(t p) c -> p t c", t=2)
        eng.dma_start(out=dst, in_=osb)

    for n2 in range(4):
        oproj2_tail(n2)


_CACHE = {}


def _get_nc():
    if "nc" in _CACHE:
        return _CACHE["nc"]
    tile.TileContext._drain_and_barrier = _drain_and_barrier_split
    nc = bass.Bass("TRN2", target_bir_lowering=False, debug=False)
    din = {
        "x8": nc.dram_tensor("x8", [128, 4, 2, S], F8,
                             kind="ExternalInput").ap(),
        "x16": nc.dram_tensor("x16", [128, 4, 2, 256], BF16,
                              kind="ExternalInput").ap(),
        "wq8": nc.dram_tensor("wq8", [128, 4, 2, 2, 2, 128], F8,
                              kind="ExternalInput").ap(),
        "wk8": nc.dram_tensor("wk8", [128, 4, 2, 2, 2, 128], F8,
                              kind="ExternalInput").ap(),
        "wv8": nc.dram_tensor("wv8", [128, 4, 2, 512], F8,
                              kind="ExternalInput").ap(),
        "wo8": nc.dram_tensor("wo8", [128, 2, 2, D], F8,
                              kind="ExternalInput").ap(),
        "wq16": nc.dram_tensor("wq16", [128, 4, 4, 2, 128], BF16,
                               kind="ExternalInput").ap(),
        "wk16": nc.dram_tensor("wk16", [128, 4, 4, 2, 128], BF16,
                               kind="ExternalInput").ap(),
        "wv16": nc.dram_tensor("wv16", [128, 4, 2, 512], BF16,
                               kind="ExternalInput").ap(),
        "wo16": nc.dram_tensor("wo16", [128, 2, 2, D], BF16,
                               kind="ExternalInput").ap(),
    }
    outT = nc.dram_tensor("outT", [D, S], F32, kind="ExternalOutput").ap()
    outT16 = nc.dram_tensor("outT16", [D, CH], BF16,
                            kind="ExternalOutput").ap()
    from contextlib import ExitStack
    with tile.TileContext(nc) as tc, ExitStack() as ctx:
        _build_kernel(ctx, tc, din, outT, outT16)
    _split_excess_waits(nc)
    _CACHE["nc"] = nc
    return nc


def make_in_maps(x, Wq, Wk, Wv, Wo):
    x = np.asarray(x, np.float32)
    Wq, Wk, Wv, Wo = (np.asarray(w, np.float32) for w in (Wq, Wk, Wv, Wo))
    xb8, xb16 = [], []
    for b in range(B):
        xT = np.ascontiguousarray(x[b].T)                      # [D, S]
        x4 = xT.reshape(4, 2, 128, S).transpose(2, 0, 1, 3)    # [128,4,2,S]
        xb8.append(np.ascontiguousarray((x4 * XS)).astype(NPF8))
        xb16.append(np.ascontiguousarray(x4[:, :, :, :256]).astype(NPBF))

    def pack_dr_qk(Wm, hh):
        # [128p, 4pr, 2csl, 2hg, 2qsl, 128(h*32+dk)]
        Wc = Wm[512 * hh:512 * hh + 512, :]
        a = Wc.reshape(2, 4, 2, 32, 4, 2, 128)   # hg,h,qsl,dk,pr,csl,p
        a = a.transpose(6, 4, 5, 0, 2, 1, 3)     # p,pr,csl,hg,qsl,h,dk
        return np.ascontiguousarray(a.reshape(128, 4, 2, 2, 2, 128))

    def pack_16_qk(Wm, hh):
        # [128p, 4mb, 4pr, 2csl, 128(a*64+dk)]
        Wc = Wm[512 * hh:512 * hh + 512, :]
        a = Wc.reshape(4, 2, 64, 4, 2, 128)      # mb,a,dk,pr,csl,p
        a = a.transpose(5, 0, 3, 4, 1, 2)        # p,mb,pr,csl,a,dk
        return np.ascontiguousarray(a.reshape(128, 4, 4, 2, 128))

    def pack_v(Wm, hh):
        # [128p, 4pr, 2csl, 512(h*64+dv)]
        Wc = Wm[512 * hh:512 * hh + 512, :]
        a = Wc.reshape(8, 64, 4, 2, 128)         # h,dv,pr,csl,p
        a = a.transpose(4, 2, 3, 0, 1)           # p,pr,csl,h,dv
        return np.ascontiguousarray(a.reshape(128, 4, 2, 512))

    def pack_o(Wm, hh):
        # [128p, 2pr, 2sl, 1024n]
        Wc = Wm[:, 512 * hh:512 * hh + 512].T    # [512 hd, 1024 n]
        a = Wc.reshape(2, 2, 128, D)             # pr,sl,p,n
        return np.ascontiguousarray(a.transpose(2, 0, 1, 3))

    packs = []
    for hh in range(2):
        packs.append({
            "wq8": (pack_dr_qk(Wq, hh) * WS).astype(NPF8),
            "wk8": (pack_dr_qk(Wk, hh) * WS).astype(NPF8),
            "wv8": (pack_v(Wv, hh) * WS).astype(NPF8),
            "wo8": (pack_o(Wo, hh) * WS).astype(NPF8),
            "wq16": pack_16_qk(Wq, hh).astype(NPBF),
            "wk16": pack_16_qk(Wk, hh).astype(NPBF),
            "wv16": pack_v(Wv, hh).astype(NPBF),
            "wo16": pack_o(Wo, hh).astype(NPBF),
        })

    in_maps = []
    for c in range(8):
        b, hh = c // 2, c % 2
        m = {"x8": xb8[b], "x16": xb16[b]}
        m.update(packs[hh])
        in_maps.append(m)
    return in_maps


def kernel(x, Wq, Wk, Wv, Wo, _trace=False, _trace_kwargs=None):
    nc = _get_nc()
    in_maps = make_in_maps(x, Wq, Wk, Wv, Wo)
    res = run_bass_kernel_spmd(
        nc, in_maps, core_ids=list(range(8)), trace=_trace,
        **(_trace_kwargs or {}))
    outs = []
    for c in range(8):
        o = res.results[c]["outT"].copy()
        o[:, 3 * CH:] = res.results[c]["outT16"].astype(np.float32)
        outs.append(o)
    full = np.stack([(outs[2 * b] + outs[2 * b + 1]).T for b in range(B)])
    if _trace:
        _CACHE["last_results"] = res
    return full.astype(np.float32)


# revision 49
# speedup vs baseline: 1.5751x; 1.0024x over previous
"""Causal multi-head self-attention on 8 Trainium2 NeuronCores.

Problem: x[4,2048,1024], Wq/Wk/Wv/Wo[1024,1024], H=16 heads, dk=64.
  q = x@Wq.T, k = x@Wk.T, v = x@Wv.T  (per-head causal softmax(q k^T/8) v) @ Wo.T

Sharding: core c handles batch b=c//2 and head-half hh=c%2 (8 heads).
Each core returns a partial transposed output outT[D,S] (its 512 attn
columns through the matching 512 rows of Wo.T); the host sums core
pairs and transposes.

Precision plan (rel-err budget 2e-2; lands ~5e-3):
  chunk 0 (q rows 0..511)   : bf16 everywhere (early rows have small
                              softmax support -> errors don't average).
  chunks 1-3 (rows 512..2047): fp8e4m3 with MatmulPerfMode.DoubleRow
                              (0.5 cycles/output column, 2x PE rate).
Host pre-quantizes and pre-packs x and all weights into the exact SBUF
tile layouts (including DoubleRow pair/slot packing and head
permutations), so the kernel DMAs everything linearly.

Kernel structure per chunk:
  scores^T [k,q] via DR matmuls (q/k stored [32-part, 2 dk-slot, s],
  4 heads per tile on PE quadrants 0/32/64/96 via explicit
  tile_position); exp on ACT (scale folds the 16*16 fp8 scaling,
  bias ln4 rescales e into fp8 range) writing fp8 e-tiles directly;
  causal masking = column trim + gpsimd affine_select zero-fill;
  PV reoriented as e.T@v -> at[q, 65] (65-column outputs; ones column
  of v gives the denominator; DR pairs 2 k-blocks per matmul);
  normalization = DVE reciprocal + per-partition tensor_scalar (the
  [q,dv] orientation makes the denominator a per-partition scalar);
  attn transposed back to [hd,q] with bf16 PE-transposes (2 heads per
  128x128 transpose) for the DoubleRow O-projection, which emits
  outT[d,q] tiles DMA'd to a transposed DRAM output.

exp on ACT is the bottleneck (~16.8M causal score elements per core at
1 elem/cycle/partition); PE work is interleaved into the exp shadow
via a filler deque (next chunk's projections, previous chunk's
transposes + O-projection).
"""

import numpy as np
import ml_dtypes

import concourse.bass as bass
import concourse.mybir as mybir
import concourse.tile as tile
from concourse.bass_utils import run_bass_kernel_spmd
from concourse.vector_clock import ScopedClock, VectorClock
from collections import deque

B, S, D, H, DK = 4, 2048, 1024, 16, 64
HPC = H // 2          # heads per core
CH = 512              # q-chunk width
NCH = S // CH         # 4
W = CH // 128         # q-windows per chunk (4)
F32 = mybir.dt.float32
I32 = mybir.dt.int32
F32R = mybir.dt.float32r
BF16 = mybir.dt.bfloat16
F8 = mybir.dt.float8e4
DR = mybir.MatmulPerfMode.DoubleRow
EXP = mybir.ActivationFunctionType.Exp
LN4 = float(np.log(4.0))
NPF8 = ml_dtypes.float8_e4m3
NPBF = ml_dtypes.bfloat16

XS, WS = 8.0, 256.0   # host scales: x8 = 8x, w8 = 256W
# Schraudolph exp2 bit-trick constants for offloaded exps (DVE+Pool):
# i32 = round(sc * 2^-11 * log2e * 2^23 + (129 - cadj) * 2^23); bitcast f32
# gives ~4*exp(sc/2048) with ~3% piecewise-linear error.
EXA = float(np.float32(1.4426950408889634 * (1 << 23) * 2.0 ** -11))
EXB = float(np.float32((129.0 - 0.044) * (1 << 23)))
# fp8-projection psum = 2048*val -> q8/k8 stored 16*val, v8 stored 32*val
QCP = 2.0 ** -7
VCP = 2.0 ** -6
OCP = 2.0 ** -13      # O-proj psum (32*256=8192) -> out


def _drain_and_barrier_split(self, tick_clock, wait_clock):
    # The stock Tile tail drain attaches every outstanding sem wait to one
    # Drain instruction; this walrus build caps sync waits per instruction
    # and rejects it.  Put each wait on its own SP nop first, then drain
    # with no waits (SP has observed everything by then).
    gc = tick_clock.global_clock
    n = len(gc)
    for proc in range(n):
        t = gc[proc]
        if t == 0:
            continue
        vc = VectorClock([0] * n)
        vc.require_at_least(proc, t)
        nop = self.nc.sync.nop(nofuse=True)
        wait_clock.add_sem_waits(nop.ins, ScopedClock({None: vc}))
    self.nc.sync.drain()
    self.nc.all_engine_barrier()
    assert self.sems is not None
    popped = self.nc._tile_sem_poison_stack.pop()
    assert popped is self._sem_poison
    self.nc.clear_and_free_semaphores(list(self.sems.allocated().values()))
    self.nc.all_engine_barrier()


def _split_excess_waits(nc, max_waits=1):
    # This walrus build rejects instructions carrying more than a couple of
    # sem waits ("Too many sync wait commands").  Engines execute their
    # stream in order, so excess waits can be moved onto nofuse nops placed
    # immediately before the instruction on the same engine.
    ctr = 0
    for blk in nc.m.functions[0].blocks:
        insts = blk.instructions
        out = []
        changed = False
        for inst in insts:
            si = inst.sync_info
            if si is not None and si.on_wait and len(si.on_wait) > max_waits:
                waits = list(si.on_wait)
                extra, keep = waits[:-max_waits], waits[-max_waits:]
                for gi in range(0, len(extra), max_waits):
                    ctr += 1
                    out.append(mybir.InstNoOp(
                        name=f"wsplit_{ctr}",
                        engine=inst.engine,
                        bass_nofuse=True,
                        sync_info=mybir.SyncInfo(
                            on_wait=extra[gi:gi + max_waits], on_update=[]),
                    ))
                inst.sync_info = mybir.SyncInfo(
                    on_wait=keep, on_update=si.on_update)
                changed = True
            out.append(inst)
        if changed:
            insts[:] = out


def _build_kernel(ctx, tc, din, outT, outT16):
    nc = tc.nc

    wpool = ctx.enter_context(tc.tile_pool(name="weights", bufs=1))
    kvpool = ctx.enter_context(tc.tile_pool(name="kv", bufs=1))
    qpool = ctx.enter_context(tc.tile_pool(name="q", bufs=2))
    epool = ctx.enter_context(tc.tile_pool(name="exp", bufs=16))
    apool = ctx.enter_context(tc.tile_pool(name="attn", bufs=2))
    opool = ctx.enter_context(tc.tile_pool(name="osb", bufs=4))
    rpool = ctx.enter_context(tc.tile_pool(name="recip", bufs=2))
    # PSUM, 8 banks: sc 2x[128,2,512]f32 (4: ACT-consumed score ring)
    #   + sco 2x[128,512]f32 (2: DVE-consumed offloaded-score ring)
    #   + at 1x[128,4,128]f32 (1: PV accumulator; single-buffered thanks to
    #     the deferred head-tail) + aux 1x (1: fillers/transposes/O-proj)
    pp = ctx.enter_context(tc.tile_pool(name="pp", bufs=2, space="PSUM"))

    # ---- persistent tiles ----
    x8 = wpool.tile([128, 4, 2, S], F8, tag="x8")
    x16 = wpool.tile([128, 4, 2, 256], BF16, tag="x16")
    wq8 = wpool.tile([128, 4, 2, 2, 2, 128], F8, tag="wq8")
    wk8 = wpool.tile([128, 4, 2, 2, 2, 128], F8, tag="wk8")
    wv8 = wpool.tile([128, 4, 2, 512], F8, tag="wv8")
    wo8 = wpool.tile([128, 2, 2, D], F8, tag="wo8")
    wq16 = [wpool.tile([128, 4, 2, 128], BF16, tag=f"wq16_{mb}",
                       name=f"wq16_{mb}") for mb in range(4)]
    wk16 = [wpool.tile([128, 4, 2, 128], BF16, tag=f"wk16_{mb}",
                       name=f"wk16_{mb}") for mb in range(4)]
    wv16 = wpool.tile([128, 4, 2, 512], BF16, tag="wv16")
    wo16 = wpool.tile([128, 2, 2, D], BF16, tag="wo16")

    kT8 = [kvpool.tile([128, 2, S], F8, tag=f"kT8_{hg}", name=f"kT8_{hg}")
           for hg in range(2)]
    kT16 = kvpool.tile([128, 4, 256], BF16, tag="kT16")
    q16 = kvpool.tile([128, 4, 256], BF16, tag="q16")
    v8 = kvpool.tile([128, 8, 2, HPC, DK + 1], F8, tag="v8")
    v16 = kvpool.tile([128, 2, 2, HPC, DK + 1], BF16, tag="v16")
    attnT8 = kvpool.tile([128, 2, 2, S], F8, tag="attnT8")
    attnT16 = kvpool.tile([128, 2, 2, CH], BF16, tag="attnT16")

    ident = wpool.tile([128, 128], BF16, tag="ident")
    ln4ap = wpool.tile([128, 1], F32, tag="ln4")
    warm = wpool.tile([128, 128], F32R, tag="warm")

    # ---- input DMAs (priority order on the sync queue) ----
    nc.sync.dma_start(out=wk8, in_=din["wk8"])
    nc.sync.dma_start(out=wq8, in_=din["wq8"])
    csq = slice(0, CH)
    nc.sync.dma_start(out=x8[:, :, :, csq], in_=din["x8"][:, :, :, csq])
    for pr in range(4):
        nc.sync.dma_start(out=x16[:, pr], in_=din["x16"][:, pr])
    nc.sync.dma_start(out=wq16[0], in_=din["wq16"][:, 0])
    nc.sync.dma_start(out=wk16[0], in_=din["wk16"][:, 0])
    nc.sync.dma_start(out=wv16, in_=din["wv16"])
    nc.sync.dma_start(out=wv8, in_=din["wv8"])
    csb = slice(CH, 2 * CH)
    nc.sync.dma_start(out=x8[:, :, :, csb], in_=din["x8"][:, :, :, csb])
    for mb in range(1, 4):
        nc.sync.dma_start(out=wq16[mb], in_=din["wq16"][:, mb])
        nc.sync.dma_start(out=wk16[mb], in_=din["wk16"][:, mb])
    csr = slice(2 * CH, S)
    nc.sync.dma_start(out=x8[:, :, :, csr], in_=din["x8"][:, :, :, csr])
    nc.sync.dma_start(out=wo8, in_=din["wo8"])
    nc.sync.dma_start(out=wo16, in_=din["wo16"])

    # ---- init + warmup (under input DMAs) ----
    warm_f = wpool.tile([128, 128], F32, tag="warm_f")
    nc.gpsimd.memset(warm_f, 0.0)
    nc.gpsimd.tensor_copy(warm, warm_f)
    tmpf = wpool.tile([128, 128], F32, tag="tmpf")
    nc.vector.memset(tmpf, 1.0)
    nc.gpsimd.affine_select(out=tmpf, in_=tmpf,
                            compare_op=mybir.AluOpType.is_equal, fill=0.0,
                            base=0, channel_multiplier=-1, pattern=[[1, 128]])
    nc.vector.tensor_copy(ident, tmpf)
    nc.vector.memset(ln4ap, LN4)
    nc.vector.memset(v8[:, :, :, :, DK:DK + 1], 1.0)
    nc.vector.memset(v16[:, :, :, :, DK:DK + 1], 1.0)
    # preload the ACT exp table set early
    etw = rpool.tile([128, 16], F32, tag="etw", bufs=1)
    nc.scalar.activation(out=etw, in_=tmpf[:, 0:16], func=EXP, scale=1.0)
    # hold the PE clock-gate open / absorb the cold ramp while DMAs land
    wps = pp.tile([128, 2, CH], F32, tag="sc", name="wps")
    for r in range(26):
        nc.tensor.matmul(wps[:, r % 2, 0:128], lhsT=warm, rhs=warm,
                         start=True, stop=True)

    # ---- chunk-0 bf16 projections ----
    # q16/kT16: baseline row layout (mb-block = 2 heads x 64 dk).
    def proj16q(mb, wtiles, dst, tag="sco"):
        def g():
            ps = pp.tile([128, 256], F32, tag=tag, bufs=None if tag == "sc"
                         else 2, name="pq16")
            for pr in range(4):
                for csl in range(2):
                    nc.tensor.matmul(
                        ps, lhsT=wtiles[mb][:, pr, csl, :],
                        rhs=x16[:, pr, csl, :],
                        start=(pr == 0 and csl == 0),
                        stop=(pr == 3 and csl == 1))
            nc.vector.tensor_copy(dst[:, mb, :], ps)
        return g

    def vproj16(sb):
        # v16 s-block sb (+ fp8 copy for later chunks)
        def f():
            ps = pp.tile([128, CH], F32, tag="sco", bufs=2, name="pv16")
            for pr in range(4):
                for csl in range(2):
                    nc.tensor.matmul(
                        ps, lhsT=x16[:, pr, csl, sb * 128:(sb + 1) * 128],
                        rhs=wv16[:, pr, csl, :],
                        start=(pr == 0 and csl == 0),
                        stop=(pr == 3 and csl == 1))
            pv = ps.rearrange("p (h d) -> p h d", h=HPC)
            nc.vector.tensor_copy(v16[:, sb // 2, sb % 2, :, 0:DK], pv)
            nc.vector.tensor_scalar(
                out=v8[:, sb // 2, sb % 2, :, 0:DK], in0=pv, scalar1=32.0,
                scalar2=None, op0=mybir.AluOpType.mult)
        return f

    def kproj8_c0(hg, sl):
        # fp8 K-projection of chunk-0 columns into kT8 (DR layout)
        def f():
            ps = pp.tile([128, CH], F32, tag="sco", bufs=2, name="pk8c0")
            for pr in range(4):
                nc.tensor.matmul(
                    ps, lhsT=wk8[:, pr, :, hg, sl, :],
                    rhs=x8[:, pr, :, 0:CH],
                    start=(pr == 0), stop=(pr == 3), perf_mode=DR)
            nc.vector.tensor_scalar(
                out=kT8[hg][:, sl, 0:CH], in0=ps, scalar1=QCP,
                scalar2=None, op0=mybir.AluOpType.mult)
        return f

    # ---- fp8 projections for chunk j>=1 ----
    def proj8(wsb, dstf, j, hg, sl, cs=None, tag="sco"):
        # q8/kT8 [4 heads x 32] dk-slot sl over chunk-j columns
        if cs is None:
            cs = slice(j * CH, (j + 1) * CH)
        def f():
            ps = pp.tile([128, cs.stop - cs.start], F32, tag=tag,
                         bufs=2 if tag == "sco" else 1, name="p8")
            for pr in range(4):
                nc.tensor.matmul(
                    ps, lhsT=wsb[:, pr, :, hg, sl, :],
                    rhs=x8[:, pr, :, cs],
                    start=(pr == 0), stop=(pr == 3), perf_mode=DR)
            nc.vector.tensor_scalar(
                out=dstf(hg, sl, cs), in0=ps, scalar1=QCP, scalar2=None,
                op0=mybir.AluOpType.mult)
        return f

    def vproj8(j, sb, tag="sco"):
        # v8 s-block 4j+sb
        def f():
            ps = pp.tile([128, CH], F32, tag=tag,
                         bufs=2 if tag == "sco" else 1, name="pv8")
            blk = 4 * j + sb
            for pr in range(4):
                nc.tensor.matmul(
                    ps, lhsT=x8[:, pr, :, blk * 128:(blk + 1) * 128],
                    rhs=wv8[:, pr, :, :],
                    start=(pr == 0), stop=(pr == 3), perf_mode=DR)
            nc.vector.tensor_scalar(
                out=v8[:, blk // 2, blk % 2, :, 0:DK],
                in0=ps.rearrange("p (h d) -> p h d", h=HPC),
                scalar1=VCP, scalar2=None, op0=mybir.AluOpType.mult)
        return f

    # ---- filler machinery ----
    fillers = deque()    # (label, fn): deadline-ordered work
    bg = deque()         # no-deadline work (O-projections)
    done = set()

    def pop_filler(n=1):
        for _ in range(n):
            if fillers:
                label, f = fillers.popleft()
                f()
                done.add(label)
            elif bg:
                label, f = bg.popleft()
                f()
                done.add(label)
            else:
                return

    def need(label):
        while label not in done and fillers:
            lb, f = fillers.popleft()
            f()
            done.add(lb)

    # ---- per-chunk attention ----
    def emit_chunk(j, q8ch):
        """Attention for chunk j.  q8ch: [128, 2hg, 2sl, CH] fp8 q tile
        (chunk 0 only holds columns 256:512; its windows 0-1 run on the
        bf16 q16/kT16/v16 path)."""
        bf = j == 0
        for h in range(HPC):
            if bf and h >= 2:
                need(f"q16_{h // 2}")
                need(f"k16_{h // 2}")
            at = pp.tile([128, W, 128], F32, tag="at", bufs=1,
                         name=f"at{j}_{h}")
            npairs = 2 * (j + 1)
            es = [None] * npairs

            def qlo(i):
                return max(0, 128 * i - CH * j)

            def emit_pv_all(h=h, at=at, es=es):
                # One window at a time: a start=True marks the whole PSUM
                # bank pending-zero, so windows must be accumulated fully
                # before the next window's group begins.
                for w in range(W):
                    gmax = (4 * j + w) // 2
                    if bf and w < 2:
                        e16 = es[0][0]
                        for t in range(w + 1):
                            nc.tensor.matmul(
                                at[:, w, 0:DK + 1],
                                lhsT=e16[:, t, 128 * w:128 * (w + 1)],
                                rhs=v16[:, 0, t, h, :],
                                start=(t == 0), stop=(t == w),
                                skip_group_check=True)
                        continue
                    for g in range(gmax + 1):
                        e8 = es[g][1] if bf else es[g]
                        nc.tensor.matmul(
                            at[:, w, 0:DK + 1],
                            lhsT=e8[:, :, 128 * w:128 * (w + 1)],
                            rhs=v8[:, g, :, h, :],
                            start=(g == 0), stop=(g == gmax),
                            perf_mode=DR, skip_group_check=True)

            gorder = list(range(npairs))
            if j == NCH - 1 and h == HPC - 1:
                # last head: diagonal (masked) pairs first so the final
                # PV burst doesn't wait on late pool-side masks
                gorder = gorder[-2:] + gorder[:-2]
            for g in gorder:
                qlp = qlo(2 * g)
                offl = ((j == 2 and g == 1)
                        or (j == 3 and g in ((3, 5) if h <= 1 else (1, 3, 5))
                            and not (h == HPC - 1 and g == 5)))
                if offl:
                    sco = [pp.tile([128, CH], F32, tag="sco", bufs=2,
                                   name=f"sco{t}") for t in range(2)]
                    sc = None
                else:
                    sc = pp.tile([128, 2, CH], F32, tag="sc", name="sc")
                # fp8 scores for columns [max(ql,256*bf):512)
                hg, hq = h // 4, h % 4
                if bf and g == 0:
                    for sl in range(2):
                        need(f"k8c0_{hg}{sl}")
                        need(f"q8c0_{hg}{sl}")
                rq = slice(32 * hq, 32 * hq + 32)
                for t in range(2):
                    i = 2 * g + t
                    ql = max(qlo(i), 256) if bf else qlo(i)
                    nc.tensor.matmul(
                        sco[t][:, ql:] if offl else sc[:, t, ql:],
                        lhsT=kT8[hg][rq, :, 128 * i:128 * (i + 1)],
                        rhs=q8ch[rq, hg, :, ql:],
                        start=True, stop=True, perf_mode=DR,
                        tile_position=(32 * hq, 0))
                if bf:
                    e8 = epool.tile([128, 2, CH], F8, tag="e8",
                                    name=f"e8_{j}_{h}_{g}")
                    q8l = max(qlp, 256)
                    nc.scalar.activation(
                        out=e8[:, :, q8l:], in_=sc[:, :, q8l:], func=EXP,
                        scale=2.0 ** -11, bias=ln4ap[:, 0:1])
                    for t in range(2):
                        i = 2 * g + t
                        hi8 = min(128 * i + 128, CH)
                        if hi8 > q8l:
                            nc.gpsimd.affine_select(
                                out=e8[:, t, q8l:hi8],
                                in_=e8[:, t, q8l:hi8],
                                compare_op=mybir.AluOpType.is_ge,
                                fill=0.0, base=q8l - i * 128,
                                channel_multiplier=-1,
                                pattern=[[1, hi8 - q8l]])
                    es[g] = (None, e8)
                else:
                    e = epool.tile([128, 2, CH], F8, tag="e8",
                                   name=f"e{j}_{h}_{g}")
                    if offl:
                        # offload to DVE (exp2 bit-trick) + Pool (fp8 cast)
                        i32 = epool.tile([128, 2, CH], I32, tag="i32",
                                         bufs=4, name=f"i{j}_{h}_{g}")
                        for t in range(2):
                            nc.vector.tensor_scalar(
                                out=i32[:, t, :], in0=sco[t], scalar1=EXA,
                                scalar2=EXB, op0=mybir.AluOpType.mult,
                                op1=mybir.AluOpType.add)
                        nc.gpsimd.tensor_copy(e, i32.bitcast(F32))
                    else:
                        nc.scalar.activation(
                            out=e[:, :, qlp:], in_=sc[:, :, qlp:], func=EXP,
                            scale=2.0 ** -11, bias=ln4ap[:, 0:1])
                    for t in range(2):
                        i = 2 * g + t
                        if i >= 4 * j:
                            hi = min(128 * (i - 4 * j) + 128, CH)
                            if hi > qlp:
                                nc.gpsimd.affine_select(
                                    out=e[:, t, qlp:hi], in_=e[:, t, qlp:hi],
                                    compare_op=mybir.AluOpType.is_ge,
                                    fill=0.0, base=j * CH + qlp - i * 128,
                                    channel_multiplier=-1,
                                    pattern=[[1, hi - qlp]])
                    es[g] = e
                if g == gorder[1] and pend_tail[0] is not None:
                    pend_tail[0]()
                    pend_tail[0] = None
                if j < 2:
                    pop_filler(2 if len(fillers) > 10 else 1)
                elif g not in gorder[:2]:
                    pop_filler(1)

            if bf:
                # bf16 scores + exp for columns [0:256) (windows 0-1),
                # emitted after the head's fp8 stream so the late-arriving
                # x16/wq16 DMAs never stall the fp8 pipeline
                need("q16_0")
                need("k16_0")
                mb, a = h // 2, h % 2
                row = slice(a * DK, (a + 1) * DK)
                sc16 = pp.tile([128, 2, 256], F32, tag="sco", bufs=2,
                               name="sc16")
                for t in range(2):
                    ql = qlo(t)
                    nc.tensor.matmul(
                        sc16[:, t, ql:],
                        lhsT=kT16[row, mb, 128 * t:128 * (t + 1)],
                        rhs=q16[row, mb, ql:],
                        start=True, stop=True)
                e16 = epool.tile([128, 2, 256], BF16, tag="e16",
                                 name=f"e16_{h}")
                nc.scalar.activation(
                    out=e16, in_=sc16, func=EXP,
                    scale=0.125, bias=ln4ap[:, 0:1])
                for t in range(2):
                    hi16 = min(128 * t + 128, 256)
                    nc.gpsimd.affine_select(
                        out=e16[:, t, 0:hi16], in_=e16[:, t, 0:hi16],
                        compare_op=mybir.AluOpType.is_ge,
                        fill=0.0, base=-t * 128,
                        channel_multiplier=-1, pattern=[[1, hi16]])
                es[0] = (e16, es[0][1])

            def head_tail(h=h, at=at, es=es, attn16_t=attn16,
                          emit_pv_all=emit_pv_all):
                if bf:
                    need("v16_0")
                    need("v16_1")
                    need("v8p_0_2")
                    need("v8p_0_3")
                emit_pv_all()
                # normalization: recip of the ones-column, then one
                # broadcast multiply over all four windows
                rc = rpool.tile([128, W, 1], F32, tag="rc",
                                name=f"rc{j}_{h}")
                with nc.allow_low_precision(
                        reason="softmax denominator recip"):
                    nc.vector.reciprocal(out=rc[:, :, 0], in_=at[:, :, DK])
                if bf:
                    # windows 2-3 carry the fp8 32x scale; fold 1/32 into rc
                    nc.vector.tensor_scalar(
                        out=rc[:, 2:4, 0], in0=rc[:, 2:4, 0],
                        scalar1=2.0 ** -5, scalar2=None,
                        op0=mybir.AluOpType.mult)
                nc.vector.tensor_tensor(
                    out=attn16_t[:, :, h, :], in0=at[:, :, 0:DK],
                    in1=rc.to_broadcast([128, W, DK]),
                    op=mybir.AluOpType.mult)
                if h % 2 == 1:
                    # transpose this head pair once both are normalized
                    fillers.append((f"tr_{j}_{h // 2}",
                                    transpose_fn(j, h // 2, attn16_t)))
            if pend_tail[0] is not None:
                pend_tail[0]()
            pend_tail[0] = head_tail

    def transpose_fn(j, hp, attn16_t):
        bf = j == 0
        def f():
            tr = pp.tile([128, W, 256], BF16, tag="aux", bufs=1, name="tr")
            for w in range(W):
                nc.tensor.matmul(
                    tr[:, w, 0:128],
                    lhsT=attn16_t[:, w, 2 * hp:2 * hp + 2, :],
                    rhs=ident, is_transpose=True, start=True, stop=True)
            dst = attnT16 if bf else attnT8
            cs = slice(0, CH) if bf else slice(j * CH, (j + 1) * CH)
            dview = dst[:, hp // 2, hp % 2, cs].rearrange(
                "p (a b) -> p a b", a=W)
            nc.vector.tensor_copy(dview, tr[:, :, 0:128])
        return f

    def oproj_fn(j, n):
        # O-projection psum is DMA'd straight to DRAM; the fp8 chunks'
        # 2^13 scale is divided out on the host.
        bf = j == 0
        def f():
            ps = pp.tile([128, CH], F32, tag="aux", bufs=1, name="pjo")
            if bf:
                for pr in range(2):
                    for sl in range(2):
                        nc.tensor.matmul(
                            ps, lhsT=wo16[:, pr, sl, 128 * n:128 * (n + 1)],
                            rhs=attnT16[:, pr, sl, :],
                            start=(pr == 0 and sl == 0),
                            stop=(pr == 1 and sl == 1))
            else:
                cs = slice(j * CH, (j + 1) * CH)
                for pr in range(2):
                    nc.tensor.matmul(
                        ps, lhsT=wo8[:, pr, :, 128 * n:128 * (n + 1)],
                        rhs=attnT8[:, pr, :, cs],
                        start=(pr == 0), stop=(pr == 1), perf_mode=DR)
            osb = opool.tile([128, CH], F32, tag="osb", name="osb")
            if bf:
                nc.vector.tensor_copy(osb, ps)
            else:
                nc.vector.tensor_scalar(out=osb, in0=ps, scalar1=OCP,
                                        scalar2=None, op0=mybir.AluOpType.mult)
            nc.sync.dma_start(
                out=outT[128 * n:128 * (n + 1), j * CH:(j + 1) * CH],
                in_=osb)
        return f

    # ---- main schedule ----
    def q8dst_fn(q8t):
        return lambda hg, sl, cs: q8t[:, hg, sl, cs]

    def q8dst_fn2(q8t):
        return lambda hg, sl, cs: q8t[:, hg, sl, :]

    def kdst_fn(hg, sl, cs):
        return kT8[hg][:, sl, cs]

    q8_c0 = qpool.tile([128, 2, 2, CH], F8, tag="q8", name="q8_0")
    for hg in range(2):
        for sl in range(2):
            fillers.append((f"k8c0_{hg}{sl}", kproj8_c0(hg, sl)))
            fillers.append((f"q8c0_{hg}{sl}",
                            proj8(wq8, q8dst_fn(q8_c0), 0, hg, sl,
                                  cs=slice(256, CH))))
        if hg == 0:
            fillers.append(("q16_0", proj16q(0, wq16, q16)))
            fillers.append(("k16_0", proj16q(0, wk16, kT16)))
    fillers.append(("v16_0", vproj16(0)))
    fillers.append(("v16_1", vproj16(1)))
    fillers.append(("v8p_0_2", vproj8(0, 2)))
    fillers.append(("v8p_0_3", vproj8(0, 3)))
    fillers.append(("q16_1", proj16q(1, wq16, q16)))
    fillers.append(("k16_1", proj16q(1, wk16, kT16)))
    fillers.append(("q16_2", proj16q(2, wq16, q16)))
    fillers.append(("k16_2", proj16q(2, wk16, kT16)))
    fillers.append(("q16_3", proj16q(3, wq16, q16)))
    fillers.append(("k16_3", proj16q(3, wk16, kT16)))

    prev_j = None     # chunk awaiting its O-projection
    pend_tail = [None]  # deferred PV+norm of the previous head
    q8_next = None
    q8_cur = q8_c0
    for j in range(NCH):
        if j + 1 < NCH:
            q8_next = qpool.tile([128, 2, 2, CH], F8, tag="q8",
                                 name=f"q8_{j + 1}")
            ptag = "sco" if j <= 1 else "aux"
            pitems = []
            for hg in range(2):
                for sl in range(2):
                    pitems.append((f"q8p_{j+1}_{hg}{sl}",
                                   proj8(wq8, q8dst_fn2(q8_next), j + 1,
                                         hg, sl, tag=ptag)))
                    pitems.append((f"k8p_{j+1}_{hg}{sl}",
                                   proj8(wk8, kdst_fn, j + 1, hg, sl,
                                         tag=ptag)))
            for sb in range(4):
                pitems.append((f"v8p_{j+1}_{sb}", vproj8(j + 1, sb,
                                                         tag=ptag)))
            if j == 0:
                fillers.extend(pitems)
            else:
                for it in reversed(pitems):
                    fillers.appendleft(it)
        attn16 = apool.tile([128, W, HPC, DK], BF16, tag="attn16",
                            name=f"attn16_{j}")
        if prev_j is not None:
            for n in range(8):
                bg.append((f"o_{prev_j}_{n}", oproj_fn(prev_j, n)))
        if j >= 1:
            for hg in range(2):
                for sl in range(2):
                    need(f"q8p_{j}_{hg}{sl}")
                    need(f"k8p_{j}_{hg}{sl}")
        emit_chunk(j, q8ch=q8_cur)
        prev_j = j
        q8_cur = q8_next
        # ensure all stragglers (e.g. v-projections) are in before next chunk
        if j + 1 < NCH:
            need(f"v8p_{j+1}_3")

    if pend_tail[0] is not None:
        pend_tail[0]()
        pend_tail[0] = None
    while fillers or bg:
        pop_filler()

    def oproj2_tail(n2):
        # last-chunk O-projection, 2 d-blocks per sc-tagged psum tile;
        # output in bf16 (halves the tail DMA; <0.2% quantization)
        j = prev_j
        ps = pp.tile([128, 2, CH], F32, tag="sc", name="pot")
        cs = slice(j * CH, (j + 1) * CH)
        for t in range(2):
            n = 2 * n2 + t
            for pr in range(2):
                nc.tensor.matmul(
                    ps[:, t, :], lhsT=wo8[:, pr, :, 128 * n:128 * (n + 1)],
                    rhs=attnT8[:, pr, :, cs],
                    start=(pr == 0), stop=(pr == 1), perf_mode=DR)
        osb = opool.tile([128, 2, CH], BF16, tag="osb2", name="osb2")
        if n2 % 2:
            nc.vector.tensor_scalar(out=osb, in0=ps, scalar1=OCP,
                                    scalar2=None, op0=mybir.AluOpType.mult)
        else:
            nc.scalar.activation(out=osb, in_=ps,
                                 func=mybir.ActivationFunctionType.Copy,
                                 scale=OCP)
        eng = nc.sync if n2 % 2 else nc.gpsimd
        dst = outT16[256 * n2:256 * (n2 + 1), :].rearrange(
            "(t p) c -> p t c", t=2)
        eng.dma_start(out=dst, in_=osb)

    for n2 in range(4):
        oproj2_tail(n2)


_CACHE = {}


def _get_nc():
    if "nc" in _CACHE:
        return _CACHE["nc"]
    tile.TileContext._drain_and_barrier = _drain_and_barrier_split
    nc = bass.Bass("TRN2", target_bir_lowering=False, debug=False)
    din = {
        "x8": nc.dram_tensor("x8", [128, 4, 2, S], F8,
                             kind="ExternalInput").ap(),
        "x16": nc.dram_tensor("x16", [128, 4, 2, 256], BF16,
                              kind="ExternalInput").ap(),
        "wq8": nc.dram_tensor("wq8", [128, 4, 2, 2, 2, 128], F8,
                              kind="ExternalInput").ap(),
        "wk8": nc.dram_tensor("wk8", [128, 4, 2, 2, 2, 128], F8,
                              kind="ExternalInput").ap(),
        "wv8": nc.dram_tensor("wv8", [128, 4, 2, 512], F8,
                              kind="ExternalInput").ap(),
        "wo8": nc.dram_tensor("wo8", [128, 2, 2, D], F8,
                              kind="ExternalInput").ap(),
        "wq16": nc.dram_tensor("wq16", [128, 4, 4, 2, 128], BF16,
                               kind="ExternalInput").ap(),
        "wk16": nc.dram_tensor("wk16", [128, 4, 4, 2, 128], BF16,
                               kind="ExternalInput").ap(),
        "wv16": nc.dram_tensor("wv16", [128, 4, 2, 512], BF16,
                               kind="ExternalInput").ap(),
        "wo16": nc.dram_tensor("wo16", [128, 2, 2, D], BF16,
                               kind="ExternalInput").ap(),
    }
    outT = nc.dram_tensor("outT", [D, S], F32, kind="ExternalOutput").ap()
    outT16 = nc.dram_tensor("outT16", [D, CH], BF16,
                            kind="ExternalOutput").ap()
    from contextlib import ExitStack
    with tile.TileContext(nc) as tc, ExitStack() as ctx:
        _build_kernel(ctx, tc, din, outT, outT16)
    _split_excess_waits(nc)
    _CACHE["nc"] = nc
    return nc


def make_in_maps(x, Wq, Wk, Wv, Wo):
    x = np.asarray(x, np.float32)
    Wq, Wk, Wv, Wo = (np.asarray(w, np.float32) for w in (Wq, Wk, Wv, Wo))
    xb8, xb16 = [], []
    for b in range(B):
        xT = np.ascontiguousarray(x[b].T)                      # [D, S]
        x4 = xT.reshape(4, 2, 128, S).transpose(2, 0, 1, 3)    # [128,4,2,S]
        xb8.append(np.ascontiguousarray((x4 * XS)).astype(NPF8))
        xb16.append(np.ascontiguousarray(x4[:, :, :, :256]).astype(NPBF))

    def pack_dr_qk(Wm, hh):
        # [128p, 4pr, 2csl, 2hg, 2qsl, 128(h*32+dk)]
        Wc = Wm[512 * hh:512 * hh + 512, :]
        a = Wc.reshape(2, 4, 2, 32, 4, 2, 128)   # hg,h,qsl,dk,pr,csl,p
        a = a.transpose(6, 4, 5, 0, 2, 1, 3)     # p,pr,csl,hg,qsl,h,dk
        return np.ascontiguousarray(a.reshape(128, 4, 2, 2, 2, 128))

    def pack_16_qk(Wm, hh):
        # [128p, 4mb, 4pr, 2csl, 128(a*64+dk)]
        Wc = Wm[512 * hh:512 * hh + 512, :]
        a = Wc.reshape(4, 2, 64, 4, 2, 128)      # mb,a,dk,pr,csl,p
        a = a.transpose(5, 0, 3, 4, 1, 2)        # p,mb,pr,csl,a,dk
        return np.ascontiguousarray(a.reshape(128, 4, 4, 2, 128))

    def pack_v(Wm, hh):
        # [128p, 4pr, 2csl, 512(h*64+dv)]
        Wc = Wm[512 * hh:512 * hh + 512, :]
        a = Wc.reshape(8, 64, 4, 2, 128)         # h,dv,pr,csl,p
        a = a.transpose(4, 2, 3, 0, 1)           # p,pr,csl,h,dv
        return np.ascontiguousarray(a.reshape(128, 4, 2, 512))

    def pack_o(Wm, hh):
        # [128p, 2pr, 2sl, 1024n]
        Wc = Wm[:, 512 * hh:512 * hh + 512].T    # [512 hd, 1024 n]
        a = Wc.reshape(2, 2, 128, D)             # pr,sl,p,n
        return np.ascontiguousarray(a.transpose(2, 0, 1, 3))

    packs = []
    for hh in range(2):
        packs.append({
            "wq8": (pack_dr_qk(Wq, hh) * WS).astype(NPF8),
            "wk8": (pack_dr_qk(Wk, hh) * WS).astype(NPF8),
            "wv8": (pack_v(Wv, hh) * WS).astype(NPF8),
            "wo8": (pack_o(Wo, hh) * WS).astype(NPF8),
            "wq16": pack_16_qk(Wq, hh).astype(NPBF),
            "wk16": pack_16_qk(Wk, hh).astype(NPBF),
            "wv16": pack_v(Wv, hh).astype(NPBF),
            "wo16": pack_o(Wo, hh).astype(NPBF),
        })

    in_maps = []
    for c in range(8):
        b, hh = c // 2, c % 2
        m = {"x8": xb8[b], "x16": xb16[b]}
        m.update(packs[hh])
        in_maps.append(m)
    return in_maps


def kernel(x, Wq, Wk, Wv, Wo, _trace=False, _trace_kwargs=None):
    nc = _get_nc()
    in_maps = make_in_maps(x, Wq, Wk, Wv, Wo)
    res = run_bass_kernel_spmd(
        nc, in_maps, core_ids=list(range(8)), trace=_trace,
        **(_trace_kwargs or {}))
    outs = []
    for c in range(8):
        o = res.results[c]["outT"].copy()
        o[:, 3 * CH:] = res.results[c]["outT16"].astype(np.float32)
        outs.append(o)
    full = np.stack([(outs[2 * b] + outs[2 * b + 1]).T for b in range(B)])
    if _trace:
        _CACHE["last_results"] = res
    return full.astype(np.float32)
